# revision 3
# baseline (speedup 1.0000x reference)
"""Trainium2 Bass kernel for music-transformer relative attention.

Shapes (hardcoded): x [2, 2048, 1024], 16 heads x 64 dims, MAXLEN == N == 2048.
Sharding: 8 cores = 2 batches x 4 head-groups (4 heads each). Each core computes
its heads' attention and a partial output projection (bf16); host sums the 4
partials per batch in fp32 and adds the bias.

Per-core pipeline (transposed scores; no PE transposes, no A PSUM->SBUF copies):
  qt/kt [64*4, N(+1)] transposed layout; V in vaug [m, 4, 65] (ones col at 64
  yields softmax denominators through the AV matmul for free).
  srel: P[p, j] = q_{n0+p-1} . e_{m0+j} per 128-row band (col j==valid hits a
  zero-padded e column -> the diagonal zero falls out of the matmul), -1e9 tail,
  written contiguously to a DRAM scratch.
  The skewed read back uses dma_start_transpose on the strided skew AP: ONE DMA
  per (head, band) lands srel^T 128-blocks side-by-side in per-(head, n-chunk)
  slot tiles SC [128, 16, 512] (transpose cost rides the otherwise idle DMA
  track). Scores: PSUM = K.Q^T (wide 512-col matmuls) + I.T @ srelT (2 of 3
  slots) or a DVE scalar_tensor_tensor add (every 3rd slot, balancing PE/DVE);
  exp on ACT writes A^T in place over srelT. The causal mask is the baked -1e9
  tail (exp -> exact 0); absent sub-blocks are Pool-memset to 0 so AV runs full
  512 wide. AV: vaug^T @ A^T accumulates out^T + row sums.
  Normalize: DVE reciprocal -> Pool partition_broadcast -> DVE multiply (no PE
  and no ACT in the chain). Out-proj from the transposed layout.

Schedule: progressive early phase (proj chunk k -> P bands/V blocks 4k..4k+3)
so attention chunk 0 starts after a quarter of the projection work; per-group
prologues (band transposes + slot memsets) emitted 2 groups ahead and
dispatched from SP so semaphore waits never block the ACT sequencer; each
group's AV matmuls are emitted interleaved into the NEXT group's score stream
(their exps are long done -> no exp->AV stall, scores never wait on AV); the
out-projection of chunk c is deferred past the first group of chunk c+1.
PE matmuls funnel cross-engine deps via dummy [1,1] matmuls (walrus 2-wait
limit).
"""

import sys

sys.path.insert(0, "/opt/trn_rl_repo")

import numpy as np
import ml_dtypes

import concourse.bass as bass
import concourse.tile as tile
from concourse import bacc
from concourse import mybir
from concourse.bass_utils import run_bass_kernel_spmd
from concourse.masks import make_identity

BF = mybir.dt.bfloat16
F32 = mybir.dt.float32
N = 2048
D = 1024
HD = 64
HPC = 4          # heads per core
DC = HPC * HD    # 256 head dims per core
NB = N // 128    # 16 row blocks
WMAX = N + 127   # max scratch row width

_CACHE = {}



def _build_nc():
    nc = bacc.Bacc()
    xT = nc.dram_tensor("xT", [D, N], BF, kind="ExternalInput")
    wqkvT = nc.dram_tensor("wqkvT", [D, 3 * DC], BF, kind="ExternalInput")
    eT = nc.dram_tensor("eT", [DC, N + 1], BF, kind="ExternalInput")
    wpT = nc.dram_tensor("wpT", [DC, D], BF, kind="ExternalInput")
    outp = nc.dram_tensor("outp", [N, D], BF, kind="ExternalOutput")
    scratch = nc.dram_tensor("scratch", [HPC * NB * 128 * WMAX], BF)
    BANDSTRIDE = NB * 128 * WMAX  # per-head stride in scratch elements

    from contextlib import ExitStack

    with tile.TileContext(nc) as tc, ExitStack() as ctx:
        pers = ctx.enter_context(tc.tile_pool(name="pers", bufs=1))
        psA = ctx.enter_context(tc.tile_pool(name="psA", bufs=6, space="PSUM"))
        psB = ctx.enter_context(tc.tile_pool(name="psB", bufs=2, space="PSUM"))
        ppool = ctx.enter_context(tc.tile_pool(name="ppool", bufs=3))
        scp = ctx.enter_context(tc.tile_pool(name="scp", bufs=3))
        oo = ctx.enter_context(tc.tile_pool(name="oo", bufs=3))
        llp = ctx.enter_context(tc.tile_pool(name="llp", bufs=3))

        # ---- persistent SBUF tensors ----
        xt = [pers.tile([128, N], BF, tag=f"xt{i}", name=f"xt{i}") for i in range(8)]
        wqkv = [pers.tile([128, 3 * DC], BF, tag=f"wqkv{i}", name=f"wqkv{i}") for i in range(8)]
        wp = [pers.tile([128, D], BF, tag=f"wp{i}", name=f"wp{i}") for i in range(2)]
        et = [pers.tile([128, N + 1], BF, tag=f"et{i}", name=f"et{i}") for i in range(2)]
        qt = [pers.tile([128, N + 1], BF, tag=f"qt{i}", name=f"qt{i}") for i in range(2)]
        kt = [pers.tile([128, N], BF, tag=f"kt{i}", name=f"kt{i}") for i in range(2)]
        vaug = [pers.tile([128, HPC, HD + 1], BF, tag=f"va{i}", name=f"va{i}") for i in range(NB)]
        aot = [pers.tile([128, N], BF, tag=f"ao{i}", name=f"ao{i}") for i in range(2)]
        ident = pers.tile([128, 128], BF, tag="ident", name="ident")
        ones = pers.tile([1, 64], F32, tag="ones", name="ones")

        make_identity(nc, ident[:])
        nc.gpsimd.memset(ones[:], 1.0)
        for g in range(2):
            nc.gpsimd.memset(qt[g][:, 0:1], 0.0)

        for i in range(8):
            nc.sync.dma_start(wqkv[i][:], wqkvT[bass.ts(i, 128), :])
            if i < 2:
                nc.sync.dma_start(xt[i][:, 0:1024], xT[bass.ts(i, 128), 0:1024])
                nc.sync.dma_start(xt[i][:, 1024:2048],
                                  xT[bass.ts(i, 128), 1024:2048])
            else:
                nc.sync.dma_start(xt[i][:], xT[bass.ts(i, 128), :])
        for g in range(2):
            nc.sync.dma_start(wp[g][:], wpT[bass.ts(g, 128), :])
            nc.sync.dma_start(et[g][:], eT[bass.ts(g, 128), :])

        # ---- projections (emitted progressively; see emit loop below) ----
        def emit_proj_chunk(nchunk):
            for g in range(2):
                ps = psA.tile([128, 512], F32, tag="mm", name="mm")
                for kc in range(8):
                    nc.tensor.matmul(
                        ps[:], wqkv[kc][:, bass.ts(g, 128)],
                        xt[kc][:, bass.ts(nchunk, 512)],
                        start=(kc == 0), stop=(kc == 7))
                nc.scalar.copy(qt[g][:, 1 + nchunk * 512:1 + (nchunk + 1) * 512], ps[:])
                ps2 = psA.tile([128, 512], F32, tag="mm", name="mm")
                for kc in range(8):
                    nc.tensor.matmul(
                        ps2[:], wqkv[kc][:, 256 + 128 * g:256 + 128 * (g + 1)],
                        xt[kc][:, bass.ts(nchunk, 512)],
                        start=(kc == 0), stop=(kc == 7))
                nc.scalar.copy(kt[g][:, bass.ts(nchunk, 512)], ps2[:])

        # ---- P bands (srel, diag-zero via padded e col) + skew bounce writes ----
        def emit_p_band(r0i):
            c_max = 128 * (r0i + 1)
            valid = c_max - 1          # data cols [0, valid); col valid = 0 (diag)
            W = c_max + 127            # scratch row stride
            m0 = N - valid             # first embedding index
            p4 = ppool.tile([128, HPC, WMAX], BF, tag="p4", name="p4")
            nc.gpsimd.memset(p4[:, :, c_max:W], -1e9)
            j = 0
            for h in range(HPC):
                g, ho = h // 2, 64 * (h % 2)
                for c0 in range(0, c_max, 512):
                    w = min(512, c_max - c0)
                    ps = psA.tile([128, 512], F32, tag="mm", name="mm")
                    nc.tensor.matmul(
                        ps[:, 0:w],
                        qt[g][ho:ho + 64, 128 * r0i:128 * r0i + 128],
                        et[g][ho:ho + 64, m0 + c0:m0 + c0 + w],
                        start=True, stop=True)
                    if j % 2 == 0:
                        nc.scalar.copy(p4[:, h, c0:c0 + w], ps[:, 0:w])
                    else:
                        nc.vector.tensor_copy(p4[:, h, c0:c0 + w], ps[:, 0:w])
                    j += 1
            base0 = r0i * 128 * WMAX
            if r0i >= 7:
                # split big band writes so transposes can slip in between
                for hp in range(2):
                    wr_ap = bass.AP(scratch, base0 + 2 * hp * BANDSTRIDE,
                                    [[W, 128], [BANDSTRIDE, 2], [1, W]])
                    nc.sync.dma_start(wr_ap, p4[:, 2 * hp:2 * hp + 2, 0:W])
            else:
                wr_ap = bass.AP(scratch, base0,
                                [[W, 128], [BANDSTRIDE, HPC], [1, W]])
                nc.sync.dma_start(wr_ap, p4[:, :, 0:W])

        # V-proj blocks interleaved with P bands: earlier band writes
        def emit_v_block(i):
            psv = psA.tile([128, HPC, HD], F32, tag="mm", name="mm")
            for kc in range(8):
                nc.tensor.matmul(
                    psv[:], xt[kc][:, bass.ts(i, 128)], wqkv[kc][:, 512:768],
                    start=(kc == 0), stop=(kc == 7))
            nc.gpsimd.memset(vaug[i][:, :, 64:65], 1.0)
            nc.vector.tensor_copy(vaug[i][:, :, 0:64], psv[:])

        # progressive early phase: proj chunk 0 -> bands 0-3 + V 0-3 ->
        # proj chunk 1 -> bands 4-7 + V 4-7 -> ... so attention chunk 0
        # can start after only a quarter of the projection work
        for nchunk in range(4):
            emit_proj_chunk(nchunk)
            for b in range(4 * nchunk, 4 * nchunk + 4):
                emit_v_block(b)
                emit_p_band(b)

        # Funnel cross-engine deps into PE's observed clock so no real
        # matmul needs >2 sync waits (walrus MM wait-slot limit).
        srcs = [et[0], et[1], qt[0], qt[1], kt[0], kt[1], wp[0], wp[1]]
        for i, src in enumerate(srcs):
            if i % 2 == 0:
                ps_d = psA.tile([1, 1], F32, tag="mm", name="mm")
            else:
                ps_d = psB.tile([1, 1], F32, tag="av", name="av")
            nc.tensor.matmul(ps_d[0:1, 0:1], src[0:1, 1:2], src[0:1, 1:2],
                             start=True, stop=True)
        for i, src in enumerate([vaug[0], vaug[NB - 1]]):
            ps_d = psB.tile([1, 1], F32, tag="av", name="av")
            nc.tensor.matmul(ps_d[0:1, 0:1], src[0:1, 0, 0:1], src[0:1, 0, 0:1],
                             start=True, stop=True)

        # ---- attention, chunk by chunk (chunk c = n cols [512c, 512c+512)) ----
        groups = [(c, h) for c in range(4) for h in range(HPC)]

        def prologue(c, h):
            # SC alloc + skewed+transposed srel band reads + absent-region zeros
            SC = scp.tile([128, NB, 512], BF, tag="sc", name="sc")
            for r0i in range(4 * c, 4 * c + 4):
                c_max = 128 * (r0i + 1)
                W = c_max + 127
                base = (h * NB + r0i) * 128 * WMAX
                rd_ap = bass.AP(scratch, base + 127,
                                [[W - 1, 128], [1, c_max]])
                off = 128 * (r0i - 4 * c)
                nc.sync.dma_start(SC[:, 0:r0i + 1, off:off + 128], rd_ap,
                                  transpose=True)
            for kb in range(4 * c + 1, 4 * c + 4):
                tr = 128 * kb - 512 * c
                nc.gpsimd.memset(SC[:, kb, 0:tr], 0.0)
            return SC

        def emit_outproj(c):
            for r0i in range(4 * c, 4 * c + 4):
                o_sb = oo.tile([128, 1024], BF, tag="osb", name="osb")
                for nch in range(2):
                    ps = psA.tile([128, 512], F32, tag="mm", name="mm")
                    for dc in range(2):
                        nc.tensor.matmul(
                            ps[:], aot[dc][:, bass.ts(r0i, 128)],
                            wp[dc][:, bass.ts(nch, 512)],
                            start=(dc == 0), stop=(dc == 1))
                    if nch == 0:
                        nc.scalar.copy(o_sb[:, 0:512], ps[:])
                    else:
                        nc.vector.tensor_copy(o_sb[:, 512:1024], ps[:])
                nc.sync.dma_start(outp[bass.ts(r0i, 128), :], o_sb[:])

        LEAD = 2
        SCs = {i: prologue(*groups[i]) for i in range(LEAD)}

        def make_av_closure(SC, ps_av, h, nblk):
            emitted = [0]

            def emit_some(k):
                # funnel Pool-memset + psB-release sems into PE order once
                if emitted[0] == 0:
                    ps_d = psB.tile([1, 1], F32, tag="av", name="av")
                    nc.tensor.matmul(ps_d[0:1, 0:1], SC[0:1, 0, 0:1],
                                     SC[0:1, 0, 0:1], start=True, stop=True)
                while emitted[0] < min(k, nblk):
                    kb = emitted[0]
                    nc.tensor.matmul(
                        ps_av[0:65, :], vaug[kb][:, h, :], SC[:, kb, 0:512],
                        start=(kb == 0), stop=(kb == nblk - 1))
                    emitted[0] += 1

            return emit_some

        def emit_norm(ps_av, g, ho, c):
            # normalize: reciprocal + Pool partition-broadcast + multiply
            linv = llp.tile([1, 512], F32, tag="linv", name="linv")
            nc.vector.reciprocal(linv[:], ps_av[64:65, :])
            lb = llp.tile([64, 512], F32, tag="lb", name="lb")
            nc.gpsimd.partition_broadcast(lb[:], linv[:], channels=64)
            nc.vector.tensor_mul(
                aot[g][ho:ho + 64, 512 * c:512 * (c + 1)],
                ps_av[0:64, :], lb[:])

        prev = None  # (emit_some, nblk, ps_av, g, ho, c)
        for gi, (c, h) in enumerate(groups):
            g, ho = h // 2, 64 * (h % 2)
            nblk = 4 * c + 4
            SC = SCs.pop(gi)
            if gi + LEAD < len(groups):
                SCs[gi + LEAD] = prologue(*groups[gi + LEAD])
            ps_av = psB.tile([128, 512], F32, tag="av", name="av")
            own_av = make_av_closure(SC, ps_av, h, nblk)

            stt_n = 3
            for kb in range(nblk):
                tr = max(0, 128 * kb - 512 * c)
                w = 512 - tr
                use_stt = stt_n > 0 and (kb % stt_n == stt_n - 1)
                ps = psA.tile([128, 512], F32, tag="mm", name="mm")
                nc.tensor.matmul(
                    ps[:, 0:w],
                    kt[g][ho:ho + 64, 128 * kb:128 * kb + 128],
                    qt[g][ho:ho + 64, 1 + 512 * c + tr:1 + 512 * (c + 1)],
                    start=True, stop=(True if use_stt else False))
                if use_stt:
                    nc.vector.scalar_tensor_tensor(
                        SC[:, kb, tr:512], ps[:, 0:w], 1.0, SC[:, kb, tr:512],
                        mybir.AluOpType.mult, mybir.AluOpType.add)
                    nc.scalar.activation(
                        SC[:, kb, tr:512], SC[:, kb, tr:512],
                        mybir.ActivationFunctionType.Exp, scale=0.125)
                else:
                    nc.tensor.matmul(
                        ps[:, 0:w], ident[:], SC[:, kb, tr:512],
                        start=False, stop=True)
                    nc.scalar.activation(
                        SC[:, kb, tr:512], ps[:, 0:w],
                        mybir.ActivationFunctionType.Exp, scale=0.125)
                # interleave the PREVIOUS group's AV matmuls: their exps are
                # long done, so neither side ever waits on the other
                if prev is not None:
                    pav, pnblk = prev[0], prev[1]
                    pav((kb + 1) * pnblk // nblk)
                if gi == len(groups) - 1 and kb >= 2:
                    # last group: interleave its own AVs (lag 2) to cut the tail
                    own_av(kb - 1)

            if prev is not None:
                pav, pnblk, pps_av, pg, pho, pc = prev
                pav(pnblk)
                emit_norm(pps_av, pg, pho, pc)
                if pc != c:
                    # previous group finished chunk pc: emit its out-proj
                    emit_outproj(pc)
            prev = (own_av, nblk, ps_av, g, ho, c)
        pav, pnblk, pps_av, pg, pho, pc = prev
        pav(pnblk)
        emit_norm(pps_av, pg, pho, pc)
        emit_outproj(3)
    nc.compile()
    return nc


def kernel(x, Wq, Wk, Wv, Wp, bp, rel_embed):
    x = np.asarray(x, np.float32)
    bf = ml_dtypes.bfloat16
    if "nc" not in _CACHE:
        _CACHE["nc"] = _build_nc()
    nc = _CACHE["nc"]

    in_maps = []
    for core in range(8):
        b, hg = core // 4, core % 4
        c0 = hg * DC
        wq_s = np.asarray(Wq)[c0:c0 + DC, :].T
        wk_s = np.asarray(Wk)[c0:c0 + DC, :].T
        wv_s = np.asarray(Wv)[c0:c0 + DC, :].T
        e_s = np.asarray(rel_embed)[:, c0:c0 + DC].T        # [DC, N]
        e_pad = np.concatenate([e_s, np.zeros((DC, 1), e_s.dtype)], axis=1)
        in_maps.append({
            "xT": np.ascontiguousarray(x[b].T).astype(bf),
            "wqkvT": np.ascontiguousarray(
                np.concatenate([wq_s, wk_s, wv_s], axis=1)).astype(bf),
            "eT": np.ascontiguousarray(e_pad).astype(bf),
            "wpT": np.ascontiguousarray(np.asarray(Wp)[:, c0:c0 + DC].T).astype(bf),
        })
    kw = dict(_CACHE.get("run_kwargs") or {})
    r = run_bass_kernel_spmd(nc, in_maps, list(range(8)), **kw)
    _CACHE["last_result"] = r
    res = r.results
    out = np.zeros((2, N, D), np.float32)
    for core in range(8):
        out[core // 4] += np.asarray(res[core]["outp"], np.float32)
    out += np.asarray(bp, np.float32)
    return out


# revision 4
# speedup vs baseline: 1.0035x; 1.0035x over previous
"""Trainium2 Bass kernel for music-transformer relative attention.

Shapes (hardcoded): x [2, 2048, 1024], 16 heads x 64 dims, MAXLEN == N == 2048.
Sharding: 8 cores = 2 batches x 4 head-groups (4 heads each). Each core computes
its heads' attention and a partial output projection (bf16); host sums the 4
partials per batch in fp32 and adds the bias.

Per-core pipeline (transposed scores; no PE transposes, no A PSUM->SBUF copies):
  qt/kt [64*4, N(+1)] transposed layout; V in vaug [m, 4, 65] (ones col at 64
  yields softmax denominators through the AV matmul for free).
  srel: P[p, j] = q_{n0+p-1} . e_{m0+j} per 128-row band (col j==valid hits a
  zero-padded e column -> the diagonal zero falls out of the matmul), -1e9 tail,
  written contiguously to a DRAM scratch.
  The skewed read back uses dma_start_transpose on the strided skew AP: ONE DMA
  per (head, band) lands srel^T 128-blocks side-by-side in per-(head, n-chunk)
  slot tiles SC [128, 16, 512] (transpose cost rides the otherwise idle DMA
  track). Scores: PSUM = K.Q^T (wide 512-col matmuls) + I.T @ srelT (2 of 3
  slots) or a DVE scalar_tensor_tensor add (every 3rd slot, balancing PE/DVE);
  exp on ACT writes A^T in place over srelT. The causal mask is the baked -1e9
  tail (exp -> exact 0); absent sub-blocks are Pool-memset to 0 so AV runs full
  512 wide. AV: vaug^T @ A^T accumulates out^T + row sums.
  Normalize: DVE reciprocal -> Pool partition_broadcast -> DVE multiply (no PE
  and no ACT in the chain). Out-proj from the transposed layout.

Schedule: progressive early phase (proj chunk k -> P bands/V blocks 4k..4k+3)
so attention chunk 0 starts after a quarter of the projection work; per-group
prologues (band transposes + slot memsets) emitted 2 groups ahead and
dispatched from SP so semaphore waits never block the ACT sequencer; each
group's AV matmuls are emitted interleaved into the NEXT group's score stream
(their exps are long done -> no exp->AV stall, scores never wait on AV); the
out-projection of chunk c is deferred past the first group of chunk c+1.
PE matmuls funnel cross-engine deps via dummy [1,1] matmuls (walrus 2-wait
limit).
"""

import sys

sys.path.insert(0, "/opt/trn_rl_repo")

import numpy as np
import ml_dtypes

import concourse.bass as bass
import concourse.tile as tile
from concourse import bacc
from concourse import mybir
from concourse.bass_utils import run_bass_kernel_spmd
from concourse.masks import make_identity

BF = mybir.dt.bfloat16
F32 = mybir.dt.float32
N = 2048
D = 1024
HD = 64
HPC = 4          # heads per core
DC = HPC * HD    # 256 head dims per core
NB = N // 128    # 16 row blocks
WMAX = N + 127   # max scratch row width

_CACHE = {}



def _build_nc():
    nc = bacc.Bacc()
    xT = nc.dram_tensor("xT", [D, N], BF, kind="ExternalInput")
    wqkvT = nc.dram_tensor("wqkvT", [D, 3 * DC], BF, kind="ExternalInput")
    eT = nc.dram_tensor("eT", [DC, N + 1], BF, kind="ExternalInput")
    wpT = nc.dram_tensor("wpT", [DC, D], BF, kind="ExternalInput")
    outp = nc.dram_tensor("outp", [N, D], BF, kind="ExternalOutput")
    scratch = nc.dram_tensor("scratch", [HPC * NB * 128 * WMAX], BF)
    BANDSTRIDE = NB * 128 * WMAX  # per-head stride in scratch elements

    from contextlib import ExitStack

    with tile.TileContext(nc) as tc, ExitStack() as ctx:
        pers = ctx.enter_context(tc.tile_pool(name="pers", bufs=1))
        psA = ctx.enter_context(tc.tile_pool(name="psA", bufs=6, space="PSUM"))
        psB = ctx.enter_context(tc.tile_pool(name="psB", bufs=2, space="PSUM"))
        ppool = ctx.enter_context(tc.tile_pool(name="ppool", bufs=3))
        scp = ctx.enter_context(tc.tile_pool(name="scp", bufs=3))
        oo = ctx.enter_context(tc.tile_pool(name="oo", bufs=3))
        llp = ctx.enter_context(tc.tile_pool(name="llp", bufs=3))

        # ---- persistent SBUF tensors ----
        xt = [pers.tile([128, N], BF, tag=f"xt{i}", name=f"xt{i}") for i in range(8)]
        wqkv = [pers.tile([128, 3 * DC], BF, tag=f"wqkv{i}", name=f"wqkv{i}") for i in range(8)]
        wp = [pers.tile([128, D], BF, tag=f"wp{i}", name=f"wp{i}") for i in range(2)]
        et = [pers.tile([128, N + 1], BF, tag=f"et{i}", name=f"et{i}") for i in range(2)]
        qt = [pers.tile([128, N + 1], BF, tag=f"qt{i}", name=f"qt{i}") for i in range(2)]
        kt = [pers.tile([128, N], BF, tag=f"kt{i}", name=f"kt{i}") for i in range(2)]
        vaug = [pers.tile([128, HPC, HD + 1], BF, tag=f"va{i}", name=f"va{i}") for i in range(NB)]
        aot = [pers.tile([128, N], BF, tag=f"ao{i}", name=f"ao{i}") for i in range(2)]
        ident = pers.tile([128, 128], BF, tag="ident", name="ident")
        ones = pers.tile([1, 64], F32, tag="ones", name="ones")

        make_identity(nc, ident[:])
        nc.gpsimd.memset(ones[:], 1.0)
        for g in range(2):
            nc.gpsimd.memset(qt[g][:, 0:1], 0.0)

        for i in range(8):
            nc.sync.dma_start(wqkv[i][:], wqkvT[bass.ts(i, 128), :])
            if i < 2:
                nc.sync.dma_start(xt[i][:, 0:1024], xT[bass.ts(i, 128), 0:1024])
                nc.sync.dma_start(xt[i][:, 1024:2048],
                                  xT[bass.ts(i, 128), 1024:2048])
            else:
                nc.sync.dma_start(xt[i][:], xT[bass.ts(i, 128), :])
        for g in range(2):
            nc.sync.dma_start(wp[g][:], wpT[bass.ts(g, 128), :])
            nc.sync.dma_start(et[g][:], eT[bass.ts(g, 128), :])

        # ---- projections (emitted progressively; see emit loop below) ----
        def emit_proj_chunk(nchunk):
            for g in range(2):
                ps = psA.tile([128, 512], F32, tag="mm", name="mm")
                for kc in range(8):
                    nc.tensor.matmul(
                        ps[:], wqkv[kc][:, bass.ts(g, 128)],
                        xt[kc][:, bass.ts(nchunk, 512)],
                        start=(kc == 0), stop=(kc == 7))
                nc.scalar.copy(qt[g][:, 1 + nchunk * 512:1 + (nchunk + 1) * 512], ps[:])
                ps2 = psA.tile([128, 512], F32, tag="mm", name="mm")
                for kc in range(8):
                    nc.tensor.matmul(
                        ps2[:], wqkv[kc][:, 256 + 128 * g:256 + 128 * (g + 1)],
                        xt[kc][:, bass.ts(nchunk, 512)],
                        start=(kc == 0), stop=(kc == 7))
                nc.scalar.copy(kt[g][:, bass.ts(nchunk, 512)], ps2[:])

        # ---- P bands (srel, diag-zero via padded e col) + skew bounce writes ----
        def emit_p_band(r0i):
            c_max = 128 * (r0i + 1)
            valid = c_max - 1          # data cols [0, valid); col valid = 0 (diag)
            W = c_max + 127            # scratch row stride
            m0 = N - valid             # first embedding index
            p4 = ppool.tile([128, HPC, WMAX], BF, tag="p4", name="p4")
            nc.gpsimd.memset(p4[:, :, c_max:W], -1e9)
            j = 0
            for h in range(HPC):
                g, ho = h // 2, 64 * (h % 2)
                for c0 in range(0, c_max, 512):
                    w = min(512, c_max - c0)
                    ps = psA.tile([128, 512], F32, tag="mm", name="mm")
                    nc.tensor.matmul(
                        ps[:, 0:w],
                        qt[g][ho:ho + 64, 128 * r0i:128 * r0i + 128],
                        et[g][ho:ho + 64, m0 + c0:m0 + c0 + w],
                        start=True, stop=True)
                    if j % 2 == 0:
                        nc.scalar.copy(p4[:, h, c0:c0 + w], ps[:, 0:w])
                    else:
                        nc.vector.tensor_copy(p4[:, h, c0:c0 + w], ps[:, 0:w])
                    j += 1
            base0 = r0i * 128 * WMAX
            if r0i >= 7:
                # split big band writes so transposes can slip in between
                for hp in range(2):
                    wr_ap = bass.AP(scratch, base0 + 2 * hp * BANDSTRIDE,
                                    [[W, 128], [BANDSTRIDE, 2], [1, W]])
                    nc.sync.dma_start(wr_ap, p4[:, 2 * hp:2 * hp + 2, 0:W])
            else:
                wr_ap = bass.AP(scratch, base0,
                                [[W, 128], [BANDSTRIDE, HPC], [1, W]])
                nc.sync.dma_start(wr_ap, p4[:, :, 0:W])

        # V-proj blocks interleaved with P bands: earlier band writes
        def emit_v_block(i):
            psv = psA.tile([128, HPC, HD], F32, tag="mm", name="mm")
            for kc in range(8):
                nc.tensor.matmul(
                    psv[:], xt[kc][:, bass.ts(i, 128)], wqkv[kc][:, 512:768],
                    start=(kc == 0), stop=(kc == 7))
            nc.gpsimd.memset(vaug[i][:, :, 64:65], 1.0)
            nc.vector.tensor_copy(vaug[i][:, :, 0:64], psv[:])

        # progressive early phase: proj chunk 0 -> bands 0-3 + V 0-3 ->
        # proj chunk 1 -> bands 4-7 + V 4-7 -> ... so attention chunk 0
        # can start after only a quarter of the projection work
        for nchunk in range(4):
            emit_proj_chunk(nchunk)
            for b in range(4 * nchunk, 4 * nchunk + 4):
                if b % 2 == 0:
                    emit_v_block(b)
                    emit_p_band(b)
                else:
                    emit_p_band(b)
                    emit_v_block(b)

        # Funnel cross-engine deps into PE's observed clock so no real
        # matmul needs >2 sync waits (walrus MM wait-slot limit).
        srcs = [et[0], et[1], qt[0], qt[1], kt[0], kt[1], wp[0], wp[1]]
        for i, src in enumerate(srcs):
            if i % 2 == 0:
                ps_d = psA.tile([1, 1], F32, tag="mm", name="mm")
            else:
                ps_d = psB.tile([1, 1], F32, tag="av", name="av")
            nc.tensor.matmul(ps_d[0:1, 0:1], src[0:1, 1:2], src[0:1, 1:2],
                             start=True, stop=True)
        for i, src in enumerate([vaug[0], vaug[NB - 1]]):
            ps_d = psB.tile([1, 1], F32, tag="av", name="av")
            nc.tensor.matmul(ps_d[0:1, 0:1], src[0:1, 0, 0:1], src[0:1, 0, 0:1],
                             start=True, stop=True)

        # ---- attention, chunk by chunk (chunk c = n cols [512c, 512c+512)) ----
        groups = [(c, h) for c in range(4) for h in range(HPC)]

        def prologue(c, h):
            # SC alloc + skewed+transposed srel band reads + absent-region zeros
            SC = scp.tile([128, NB, 512], BF, tag="sc", name="sc")
            for r0i in range(4 * c, 4 * c + 4):
                c_max = 128 * (r0i + 1)
                W = c_max + 127
                base = (h * NB + r0i) * 128 * WMAX
                rd_ap = bass.AP(scratch, base + 127,
                                [[W - 1, 128], [1, c_max]])
                off = 128 * (r0i - 4 * c)
                nc.sync.dma_start(SC[:, 0:r0i + 1, off:off + 128], rd_ap,
                                  transpose=True)
            for kb in range(4 * c + 1, 4 * c + 4):
                tr = 128 * kb - 512 * c
                nc.gpsimd.memset(SC[:, kb, 0:tr], 0.0)
            return SC

        def emit_outproj(c):
            for r0i in range(4 * c, 4 * c + 4):
                o_sb = oo.tile([128, 1024], BF, tag="osb", name="osb")
                for nch in range(2):
                    ps = psA.tile([128, 512], F32, tag="mm", name="mm")
                    for dc in range(2):
                        nc.tensor.matmul(
                            ps[:], aot[dc][:, bass.ts(r0i, 128)],
                            wp[dc][:, bass.ts(nch, 512)],
                            start=(dc == 0), stop=(dc == 1))
                    if nch == 0:
                        nc.scalar.copy(o_sb[:, 0:512], ps[:])
                    else:
                        nc.vector.tensor_copy(o_sb[:, 512:1024], ps[:])
                nc.sync.dma_start(outp[bass.ts(r0i, 128), :], o_sb[:])

        LEAD = 2
        SCs = {i: prologue(*groups[i]) for i in range(LEAD)}

        def make_av_closure(SC, ps_av, h, nblk):
            emitted = [0]

            def emit_some(k):
                # funnel Pool-memset + psB-release sems into PE order once
                if emitted[0] == 0:
                    ps_d = psB.tile([1, 1], F32, tag="av", name="av")
                    nc.tensor.matmul(ps_d[0:1, 0:1], SC[0:1, 0, 0:1],
                                     SC[0:1, 0, 0:1], start=True, stop=True)
                while emitted[0] < min(k, nblk):
                    kb = emitted[0]
                    nc.tensor.matmul(
                        ps_av[0:65, :], vaug[kb][:, h, :], SC[:, kb, 0:512],
                        start=(kb == 0), stop=(kb == nblk - 1))
                    emitted[0] += 1

            return emit_some

        def emit_norm(ps_av, g, ho, c):
            # normalize: reciprocal + Pool partition-broadcast + multiply
            linv = llp.tile([1, 512], F32, tag="linv", name="linv")
            nc.vector.reciprocal(linv[:], ps_av[64:65, :])
            lb = llp.tile([64, 512], F32, tag="lb", name="lb")
            nc.gpsimd.partition_broadcast(lb[:], linv[:], channels=64)
            nc.vector.tensor_mul(
                aot[g][ho:ho + 64, 512 * c:512 * (c + 1)],
                ps_av[0:64, :], lb[:])

        prev = None  # (emit_some, nblk, ps_av, g, ho, c)
        for gi, (c, h) in enumerate(groups):
            g, ho = h // 2, 64 * (h % 2)
            nblk = 4 * c + 4
            SC = SCs.pop(gi)
            if gi + LEAD < len(groups):
                SCs[gi + LEAD] = prologue(*groups[gi + LEAD])
            ps_av = psB.tile([128, 512], F32, tag="av", name="av")
            own_av = make_av_closure(SC, ps_av, h, nblk)

            stt_n = 3
            for kb in range(nblk):
                tr = max(0, 128 * kb - 512 * c)
                w = 512 - tr
                use_stt = stt_n > 0 and (kb % stt_n == stt_n - 1)
                ps = psA.tile([128, 512], F32, tag="mm", name="mm")
                nc.tensor.matmul(
                    ps[:, 0:w],
                    kt[g][ho:ho + 64, 128 * kb:128 * kb + 128],
                    qt[g][ho:ho + 64, 1 + 512 * c + tr:1 + 512 * (c + 1)],
                    start=True, stop=(True if use_stt else False))
                if use_stt:
                    nc.vector.scalar_tensor_tensor(
                        SC[:, kb, tr:512], ps[:, 0:w], 1.0, SC[:, kb, tr:512],
                        mybir.AluOpType.mult, mybir.AluOpType.add)
                    nc.scalar.activation(
                        SC[:, kb, tr:512], SC[:, kb, tr:512],
                        mybir.ActivationFunctionType.Exp, scale=0.125)
                else:
                    nc.tensor.matmul(
                        ps[:, 0:w], ident[:], SC[:, kb, tr:512],
                        start=False, stop=True)
                    nc.scalar.activation(
                        SC[:, kb, tr:512], ps[:, 0:w],
                        mybir.ActivationFunctionType.Exp, scale=0.125)
                # interleave the PREVIOUS group's AV matmuls: their exps are
                # long done, so neither side ever waits on the other
                if prev is not None:
                    pav, pnblk = prev[0], prev[1]
                    pav((kb + 1) * pnblk // nblk)
                if gi == len(groups) - 1 and kb >= 2:
                    # last group: interleave its own AVs (lag 2) to cut the tail
                    own_av(kb - 1)

            if prev is not None:
                pav, pnblk, pps_av, pg, pho, pc = prev
                pav(pnblk)
                emit_norm(pps_av, pg, pho, pc)
                if pc != c:
                    # previous group finished chunk pc: emit its out-proj
                    emit_outproj(pc)
            prev = (own_av, nblk, ps_av, g, ho, c)
        pav, pnblk, pps_av, pg, pho, pc = prev
        pav(pnblk)
        emit_norm(pps_av, pg, pho, pc)
        emit_outproj(3)
    nc.compile()
    return nc


def kernel(x, Wq, Wk, Wv, Wp, bp, rel_embed):
    x = np.asarray(x, np.float32)
    bf = ml_dtypes.bfloat16
    if "nc" not in _CACHE:
        _CACHE["nc"] = _build_nc()
    nc = _CACHE["nc"]

    in_maps = []
    for core in range(8):
        b, hg = core // 4, core % 4
        c0 = hg * DC
        wq_s = np.asarray(Wq)[c0:c0 + DC, :].T
        wk_s = np.asarray(Wk)[c0:c0 + DC, :].T
        wv_s = np.asarray(Wv)[c0:c0 + DC, :].T
        e_s = np.asarray(rel_embed)[:, c0:c0 + DC].T        # [DC, N]
        e_pad = np.concatenate([e_s, np.zeros((DC, 1), e_s.dtype)], axis=1)
        in_maps.append({
            "xT": np.ascontiguousarray(x[b].T).astype(bf),
            "wqkvT": np.ascontiguousarray(
                np.concatenate([wq_s, wk_s, wv_s], axis=1)).astype(bf),
            "eT": np.ascontiguousarray(e_pad).astype(bf),
            "wpT": np.ascontiguousarray(np.asarray(Wp)[:, c0:c0 + DC].T).astype(bf),
        })
    kw = dict(_CACHE.get("run_kwargs") or {})
    r = run_bass_kernel_spmd(nc, in_maps, list(range(8)), **kw)
    _CACHE["last_result"] = r
    res = r.results
    out = np.zeros((2, N, D), np.float32)
    for core in range(8):
        out[core // 4] += np.asarray(res[core]["outp"], np.float32)
    out += np.asarray(bp, np.float32)
    return out


# revision 6
# speedup vs baseline: 1.0317x; 1.0281x over previous
"""Trainium2 Bass kernel for music-transformer relative attention.

Shapes (hardcoded): x [2, 2048, 1024], 16 heads x 64 dims, MAXLEN == N == 2048.
Sharding: 8 cores = 2 batches x 4 head-groups (4 heads each). Each core computes
its heads' attention and a partial output projection (bf16); host sums the 4
partials per batch in fp32 and adds the bias.

Per-core pipeline (transposed scores; no PE transposes, no A PSUM->SBUF copies):
  qt/kt [64*4, N(+1)] transposed layout; V in vaug [m, 4, 65] (ones col at 64
  yields softmax denominators through the AV matmul for free).
  srel: P[p, j] = q_{n0+p-1} . e_{m0+j} per 128-row band (col j==valid hits a
  zero-padded e column -> the diagonal zero falls out of the matmul), -1e9 tail,
  written contiguously to a DRAM scratch.
  The skewed read back uses dma_start_transpose on the strided skew AP: ONE DMA
  per (head, band) lands srel^T 128-blocks side-by-side in per-(head, n-chunk)
  slot tiles SC [128, 16, 512] (transpose cost rides the otherwise idle DMA
  track). Scores: PSUM = K.Q^T (wide 512-col matmuls) + I.T @ srelT (2 of 3
  slots) or a DVE scalar_tensor_tensor add (every 3rd slot, balancing PE/DVE);
  exp on ACT writes A^T in place over srelT. The causal mask is the baked -1e9
  tail (exp -> exact 0); absent sub-blocks are Pool-memset to 0 so AV runs full
  512 wide. AV: vaug^T @ A^T accumulates out^T + row sums.
  Normalize: DVE reciprocal -> Pool partition_broadcast -> DVE multiply (no PE
  and no ACT in the chain). Out-proj from the transposed layout.

Schedule: progressive early phase (proj chunk k -> P bands/V blocks 4k..4k+3)
so attention chunk 0 starts after a quarter of the projection work; per-group
prologues (band transposes + slot memsets) emitted 2 groups ahead and
dispatched from SP so semaphore waits never block the ACT sequencer; each
group's AV matmuls are emitted interleaved into the NEXT group's score stream
(their exps are long done -> no exp->AV stall, scores never wait on AV); the
out-projection of chunk c is deferred past the first group of chunk c+1.
PE matmuls funnel cross-engine deps via dummy [1,1] matmuls (walrus 2-wait
limit).
"""

import sys

sys.path.insert(0, "/opt/trn_rl_repo")

import numpy as np
import ml_dtypes

import concourse.bass as bass
import concourse.tile as tile
from concourse import bacc
from concourse import mybir
from concourse.bass_utils import run_bass_kernel_spmd
from concourse.masks import make_identity

BF = mybir.dt.bfloat16
F32 = mybir.dt.float32
F8 = mybir.dt.float8e4
WSCALE = 32.0    # host pre-scales projection weights into fp8e4m3 range
N = 2048
D = 1024
HD = 64
HPC = 4          # heads per core
DC = HPC * HD    # 256 head dims per core
NB = N // 128    # 16 row blocks
WMAX = N + 127   # max scratch row width

_CACHE = {}



def _build_nc():
    nc = bacc.Bacc()
    # fp8 value+residual pairs, DoubleRow pair-packed: [ver, kcp, p, 2, cols]
    xp = nc.dram_tensor("xp", [2, 4, 128, 2, N], F8, kind="ExternalInput")
    wqkvp = nc.dram_tensor("wqkvp", [2, 4, 128, 2, 3 * DC], F8,
                           kind="ExternalInput")
    eT = nc.dram_tensor("eT", [DC, N + 1], BF, kind="ExternalInput")
    wpT = nc.dram_tensor("wpT", [DC, D], BF, kind="ExternalInput")
    outp = nc.dram_tensor("outp", [N, D], BF, kind="ExternalOutput")
    scratch = nc.dram_tensor("scratch", [HPC * NB * 128 * WMAX], BF)
    BANDSTRIDE = NB * 128 * WMAX  # per-head stride in scratch elements

    from contextlib import ExitStack

    with tile.TileContext(nc) as tc, ExitStack() as ctx:
        pers = ctx.enter_context(tc.tile_pool(name="pers", bufs=1))
        psA = ctx.enter_context(tc.tile_pool(name="psA", bufs=6, space="PSUM"))
        psB = ctx.enter_context(tc.tile_pool(name="psB", bufs=2, space="PSUM"))
        ppool = ctx.enter_context(tc.tile_pool(name="ppool", bufs=3))
        scp = ctx.enter_context(tc.tile_pool(name="scp", bufs=3))
        oo = ctx.enter_context(tc.tile_pool(name="oo", bufs=3))
        llp = ctx.enter_context(tc.tile_pool(name="llp", bufs=3))

        # ---- persistent SBUF tensors ----
        xt = [[pers.tile([128, 2, N], F8, tag=f"xt{v}_{i}", name=f"xt{v}_{i}")
               for i in range(4)] for v in range(2)]
        wqkv = [[pers.tile([128, 2, 3 * DC], F8, tag=f"wq{v}_{i}",
                           name=f"wq{v}_{i}") for i in range(4)] for v in range(2)]
        wp = [pers.tile([128, D], BF, tag=f"wp{i}", name=f"wp{i}") for i in range(2)]
        et = [pers.tile([128, N + 1], BF, tag=f"et{i}", name=f"et{i}") for i in range(2)]
        qt = [pers.tile([128, N + 1], BF, tag=f"qt{i}", name=f"qt{i}") for i in range(2)]
        kt = [pers.tile([128, N], BF, tag=f"kt{i}", name=f"kt{i}") for i in range(2)]
        vaug = [pers.tile([128, HPC, HD + 1], BF, tag=f"va{i}", name=f"va{i}") for i in range(NB)]
        aot = [pers.tile([128, N], BF, tag=f"ao{i}", name=f"ao{i}") for i in range(2)]
        ident = pers.tile([128, 128], BF, tag="ident", name="ident")
        ones = pers.tile([1, 64], F32, tag="ones", name="ones")

        make_identity(nc, ident[:])
        nc.gpsimd.memset(ones[:], 1.0)
        for g in range(2):
            nc.gpsimd.memset(qt[g][:, 0:1], 0.0)

        for i in range(4):
            nc.sync.dma_start(wqkv[0][i][:], wqkvp[0, i, :, :, :])
            nc.sync.dma_start(xt[0][i][:], xp[0, i, :, :, :])
        for i in range(4):
            nc.sync.dma_start(wqkv[1][i][:], wqkvp[1, i, :, :, :])
            nc.sync.dma_start(xt[1][i][:], xp[1, i, :, :, :])
        for g in range(2):
            nc.sync.dma_start(wp[g][:], wpT[bass.ts(g, 128), :])
            nc.sync.dma_start(et[g][:], eT[bass.ts(g, 128), :])

        # ---- projections: error-compensated double-fp8 DoubleRow ----
        # q = x1.W1 + x1.W2 + x2.W1 (residual^2 term dropped), K=256/matmul
        DR = mybir.MatmulPerfMode.DoubleRow
        PASSES = [(0, 0), (0, 1), (1, 0)]

        def emit_proj_chunk(nchunk):
            for g in range(2):
                ps = psA.tile([128, 512], F32, tag="mm", name="mm")
                for pi, (xv, wv) in enumerate(PASSES):
                    for kc in range(4):
                        nc.tensor.matmul(
                            ps[:], wqkv[wv][kc][:, :, bass.ts(g, 128)],
                            xt[xv][kc][:, :, bass.ts(nchunk, 512)],
                            start=(pi == 0 and kc == 0),
                            stop=(pi == 2 and kc == 3), perf_mode=DR)
                nc.scalar.mul(qt[g][:, 1 + nchunk * 512:1 + (nchunk + 1) * 512],
                              ps[:], 1.0 / WSCALE)
                ps2 = psA.tile([128, 512], F32, tag="mm", name="mm")
                for pi, (xv, wv) in enumerate(PASSES):
                    for kc in range(4):
                        nc.tensor.matmul(
                            ps2[:],
                            wqkv[wv][kc][:, :, 256 + 128 * g:256 + 128 * (g + 1)],
                            xt[xv][kc][:, :, bass.ts(nchunk, 512)],
                            start=(pi == 0 and kc == 0),
                            stop=(pi == 2 and kc == 3), perf_mode=DR)
                nc.scalar.mul(kt[g][:, bass.ts(nchunk, 512)], ps2[:],
                              1.0 / WSCALE)

        # ---- P bands (srel, diag-zero via padded e col) + skew bounce writes ----
        def emit_p_band(r0i):
            c_max = 128 * (r0i + 1)
            valid = c_max - 1          # data cols [0, valid); col valid = 0 (diag)
            W = c_max + 127            # scratch row stride
            m0 = N - valid             # first embedding index
            p4 = ppool.tile([128, HPC, WMAX], BF, tag="p4", name="p4")
            nc.gpsimd.memset(p4[:, :, c_max:W], -1e9)
            j = 0
            for h in range(HPC):
                g, ho = h // 2, 64 * (h % 2)
                for c0 in range(0, c_max, 512):
                    w = min(512, c_max - c0)
                    ps = psA.tile([128, 512], F32, tag="mm", name="mm")
                    nc.tensor.matmul(
                        ps[:, 0:w],
                        qt[g][ho:ho + 64, 128 * r0i:128 * r0i + 128],
                        et[g][ho:ho + 64, m0 + c0:m0 + c0 + w],
                        start=True, stop=True)
                    if j % 2 == 0:
                        nc.scalar.copy(p4[:, h, c0:c0 + w], ps[:, 0:w])
                    else:
                        nc.vector.tensor_copy(p4[:, h, c0:c0 + w], ps[:, 0:w])
                    j += 1
            base0 = r0i * 128 * WMAX
            if r0i >= 7:
                # split big band writes so transposes can slip in between
                for hp in range(2):
                    wr_ap = bass.AP(scratch, base0 + 2 * hp * BANDSTRIDE,
                                    [[W, 128], [BANDSTRIDE, 2], [1, W]])
                    nc.sync.dma_start(wr_ap, p4[:, 2 * hp:2 * hp + 2, 0:W])
            else:
                wr_ap = bass.AP(scratch, base0,
                                [[W, 128], [BANDSTRIDE, HPC], [1, W]])
                nc.sync.dma_start(wr_ap, p4[:, :, 0:W])

        # V-proj blocks interleaved with P bands: earlier band writes
        def emit_v_block(i):
            psv = psA.tile([128, HPC, HD], F32, tag="mm", name="mm")
            for pi, (xv, wv) in enumerate(PASSES):
                for kc in range(4):
                    nc.tensor.matmul(
                        psv[:], xt[xv][kc][:, :, bass.ts(i, 128)],
                        wqkv[wv][kc][:, :, 512:768],
                        start=(pi == 0 and kc == 0),
                        stop=(pi == 2 and kc == 3), perf_mode=DR)
            nc.gpsimd.memset(vaug[i][:, :, 64:65], 1.0)
            nc.vector.tensor_scalar_mul(vaug[i][:, :, 0:64], psv[:],
                                        1.0 / WSCALE)

        # progressive early phase: proj chunk 0 -> bands 0-3 + V 0-3 ->
        # proj chunk 1 -> bands 4-7 + V 4-7 -> ... so attention chunk 0
        # can start after only a quarter of the projection work
        for nchunk in range(4):
            emit_proj_chunk(nchunk)
            for b in range(4 * nchunk, 4 * nchunk + 4):
                if b % 2 == 1:
                    emit_v_block(b)
                    emit_p_band(b)
                else:
                    emit_p_band(b)
                    emit_v_block(b)

        # Funnel cross-engine deps into PE's observed clock so no real
        # matmul needs >2 sync waits (walrus MM wait-slot limit).
        srcs = [et[0], et[1], qt[0], qt[1], kt[0], kt[1], wp[0], wp[1]]
        for i, src in enumerate(srcs):
            if i % 2 == 0:
                ps_d = psA.tile([1, 1], F32, tag="mm", name="mm")
            else:
                ps_d = psB.tile([1, 1], F32, tag="av", name="av")
            nc.tensor.matmul(ps_d[0:1, 0:1], src[0:1, 1:2], src[0:1, 1:2],
                             start=True, stop=True)
        for i, src in enumerate([vaug[0], vaug[NB - 1]]):
            ps_d = psB.tile([1, 1], F32, tag="av", name="av")
            nc.tensor.matmul(ps_d[0:1, 0:1], src[0:1, 0, 0:1], src[0:1, 0, 0:1],
                             start=True, stop=True)

        # ---- attention, chunk by chunk (chunk c = n cols [512c, 512c+512)) ----
        groups = [(c, h) for c in range(4) for h in range(HPC)]

        def prologue(c, h):
            # SC alloc + skewed+transposed srel band reads + absent-region zeros
            SC = scp.tile([128, NB, 512], BF, tag="sc", name="sc")
            for r0i in range(4 * c, 4 * c + 4):
                c_max = 128 * (r0i + 1)
                W = c_max + 127
                base = (h * NB + r0i) * 128 * WMAX
                rd_ap = bass.AP(scratch, base + 127,
                                [[W - 1, 128], [1, c_max]])
                off = 128 * (r0i - 4 * c)
                nc.sync.dma_start(SC[:, 0:r0i + 1, off:off + 128], rd_ap,
                                  transpose=True)
            for kb in range(4 * c + 1, 4 * c + 4):
                tr = 128 * kb - 512 * c
                nc.gpsimd.memset(SC[:, kb, 0:tr], 0.0)
            return SC

        def emit_outproj(c):
            for r0i in range(4 * c, 4 * c + 4):
                o_sb = oo.tile([128, 1024], BF, tag="osb", name="osb")
                for nch in range(2):
                    ps = psA.tile([128, 512], F32, tag="mm", name="mm")
                    for dc in range(2):
                        nc.tensor.matmul(
                            ps[:], aot[dc][:, bass.ts(r0i, 128)],
                            wp[dc][:, bass.ts(nch, 512)],
                            start=(dc == 0), stop=(dc == 1))
                    if nch == 0:
                        nc.scalar.copy(o_sb[:, 0:512], ps[:])
                    else:
                        nc.vector.tensor_copy(o_sb[:, 512:1024], ps[:])
                nc.sync.dma_start(outp[bass.ts(r0i, 128), :], o_sb[:])

        LEAD = 2
        SCs = {i: prologue(*groups[i]) for i in range(LEAD)}

        def make_av_closure(SC, ps_av, h, nblk):
            emitted = [0]

            def emit_some(k):
                # funnel Pool-memset + psB-release sems into PE order once
                if emitted[0] == 0:
                    ps_d = psB.tile([1, 1], F32, tag="av", name="av")
                    nc.tensor.matmul(ps_d[0:1, 0:1], SC[0:1, 0, 0:1],
                                     SC[0:1, 0, 0:1], start=True, stop=True)
                while emitted[0] < min(k, nblk):
                    kb = emitted[0]
                    nc.tensor.matmul(
                        ps_av[0:65, :], vaug[kb][:, h, :], SC[:, kb, 0:512],
                        start=(kb == 0), stop=(kb == nblk - 1))
                    emitted[0] += 1

            return emit_some

        def emit_norm(ps_av, g, ho, c):
            # normalize: reciprocal + Pool partition-broadcast + multiply
            linv = llp.tile([1, 512], F32, tag="linv", name="linv")
            nc.vector.reciprocal(linv[:], ps_av[64:65, :])
            lb = llp.tile([64, 512], F32, tag="lb", name="lb")
            nc.gpsimd.partition_broadcast(lb[:], linv[:], channels=64)
            nc.vector.tensor_mul(
                aot[g][ho:ho + 64, 512 * c:512 * (c + 1)],
                ps_av[0:64, :], lb[:])

        prev = None  # (emit_some, nblk, ps_av, g, ho, c)
        for gi, (c, h) in enumerate(groups):
            g, ho = h // 2, 64 * (h % 2)
            nblk = 4 * c + 4
            SC = SCs.pop(gi)
            if gi + LEAD < len(groups):
                SCs[gi + LEAD] = prologue(*groups[gi + LEAD])
            ps_av = psB.tile([128, 512], F32, tag="av", name="av")
            own_av = make_av_closure(SC, ps_av, h, nblk)

            stt_n = 3
            for kb in range(nblk):
                tr = max(0, 128 * kb - 512 * c)
                w = 512 - tr
                use_stt = stt_n > 0 and (kb % stt_n == stt_n - 1)
                ps = psA.tile([128, 512], F32, tag="mm", name="mm")
                nc.tensor.matmul(
                    ps[:, 0:w],
                    kt[g][ho:ho + 64, 128 * kb:128 * kb + 128],
                    qt[g][ho:ho + 64, 1 + 512 * c + tr:1 + 512 * (c + 1)],
                    start=True, stop=(True if use_stt else False))
                if use_stt:
                    nc.vector.scalar_tensor_tensor(
                        SC[:, kb, tr:512], ps[:, 0:w], 1.0, SC[:, kb, tr:512],
                        mybir.AluOpType.mult, mybir.AluOpType.add)
                    nc.scalar.activation(
                        SC[:, kb, tr:512], SC[:, kb, tr:512],
                        mybir.ActivationFunctionType.Exp, scale=0.125)
                else:
                    nc.tensor.matmul(
                        ps[:, 0:w], ident[:], SC[:, kb, tr:512],
                        start=False, stop=True)
                    nc.scalar.activation(
                        SC[:, kb, tr:512], ps[:, 0:w],
                        mybir.ActivationFunctionType.Exp, scale=0.125)
                # interleave the PREVIOUS group's AV matmuls: their exps are
                # long done, so neither side ever waits on the other
                if prev is not None:
                    pav, pnblk = prev[0], prev[1]
                    pav((kb + 1) * pnblk // nblk)
                if gi == len(groups) - 1 and kb >= 2:
                    # last group: interleave its own AVs (lag 2) to cut the tail
                    own_av(kb - 1)

            if prev is not None:
                pav, pnblk, pps_av, pg, pho, pc = prev
                pav(pnblk)
                emit_norm(pps_av, pg, pho, pc)
                if pc != c:
                    # previous group finished chunk pc: emit its out-proj
                    emit_outproj(pc)
            prev = (own_av, nblk, ps_av, g, ho, c)
        pav, pnblk, pps_av, pg, pho, pc = prev
        pav(pnblk)
        emit_norm(pps_av, pg, pho, pc)
        emit_outproj(3)
    nc.compile()
    return nc


def kernel(x, Wq, Wk, Wv, Wp, bp, rel_embed):
    x = np.asarray(x, np.float32)
    bf = ml_dtypes.bfloat16
    if "nc" not in _CACHE:
        _CACHE["nc"] = _build_nc()
    nc = _CACHE["nc"]

    in_maps = []
    for core in range(8):
        b, hg = core // 4, core % 4
        c0 = hg * DC
        wq_s = np.asarray(Wq)[c0:c0 + DC, :].T
        wk_s = np.asarray(Wk)[c0:c0 + DC, :].T
        wv_s = np.asarray(Wv)[c0:c0 + DC, :].T
        e_s = np.asarray(rel_embed)[:, c0:c0 + DC].T        # [DC, N]
        e_pad = np.concatenate([e_s, np.zeros((DC, 1), e_s.dtype)], axis=1)
        f8 = ml_dtypes.float8_e4m3

        def pack(a):
            # [D, cols] -> [kcp, p, 2, cols]
            return np.ascontiguousarray(
                a.reshape(4, 2, 128, a.shape[1]).transpose(0, 2, 1, 3))

        xT_f = np.ascontiguousarray(x[b].T).astype(np.float32)
        x1 = xT_f.astype(f8)
        x2 = (xT_f - x1.astype(np.float32)).astype(f8)
        wcat = np.concatenate([wq_s, wk_s, wv_s], axis=1).astype(np.float32)
        wcat = wcat * WSCALE
        w1 = wcat.astype(f8)
        w2 = (wcat - w1.astype(np.float32)).astype(f8)
        in_maps.append({
            "xp": np.stack([pack(x1), pack(x2)]),
            "wqkvp": np.stack([pack(w1), pack(w2)]),
            "eT": np.ascontiguousarray(e_pad).astype(bf),
            "wpT": np.ascontiguousarray(np.asarray(Wp)[:, c0:c0 + DC].T).astype(bf),
        })
    kw = dict(_CACHE.get("run_kwargs") or {})
    r = run_bass_kernel_spmd(nc, in_maps, list(range(8)), **kw)
    _CACHE["last_result"] = r
    res = r.results
    out = np.zeros((2, N, D), np.float32)
    for core in range(8):
        out[core // 4] += np.asarray(res[core]["outp"], np.float32)
    out += np.asarray(bp, np.float32)
    return out


# revision 7
# speedup vs baseline: 1.0320x; 1.0003x over previous
"""Trainium2 Bass kernel for music-transformer relative attention.

Shapes (hardcoded): x [2, 2048, 1024], 16 heads x 64 dims, MAXLEN == N == 2048.
Sharding: 8 cores = 2 batches x 4 head-groups (4 heads each). Each core computes
its heads' attention and a partial output projection (bf16); host sums the 4
partials per batch in fp32 and adds the bias.

Per-core pipeline (transposed scores; no PE transposes, no A PSUM->SBUF copies):
  qt/kt [64*4, N(+1)] transposed layout; V in vaug [m, 4, 65] (ones col at 64
  yields softmax denominators through the AV matmul for free).
  srel: P[p, j] = q_{n0+p-1} . e_{m0+j} per 128-row band (col j==valid hits a
  zero-padded e column -> the diagonal zero falls out of the matmul), -1e9 tail,
  written contiguously to a DRAM scratch.
  The skewed read back uses dma_start_transpose on the strided skew AP: ONE DMA
  per (head, band) lands srel^T 128-blocks side-by-side in per-(head, n-chunk)
  slot tiles SC [128, 16, 512] (transpose cost rides the otherwise idle DMA
  track). Scores: PSUM = K.Q^T (wide 512-col matmuls) + I.T @ srelT (2 of 3
  slots) or a DVE scalar_tensor_tensor add (every 3rd slot, balancing PE/DVE);
  exp on ACT writes A^T in place over srelT. The causal mask is the baked -1e9
  tail (exp -> exact 0); absent sub-blocks are Pool-memset to 0 so AV runs full
  512 wide. AV: vaug^T @ A^T accumulates out^T + row sums.
  Normalize: DVE reciprocal -> Pool partition_broadcast -> DVE multiply (no PE
  and no ACT in the chain). Out-proj from the transposed layout.

Schedule: progressive early phase (proj chunk k -> P bands/V blocks 4k..4k+3)
so attention chunk 0 starts after a quarter of the projection work; per-group
prologues (band transposes + slot memsets) emitted 2 groups ahead and
dispatched from SP so semaphore waits never block the ACT sequencer; each
group's AV matmuls are emitted interleaved into the NEXT group's score stream
(their exps are long done -> no exp->AV stall, scores never wait on AV); the
out-projection of chunk c is deferred past the first group of chunk c+1.
PE matmuls funnel cross-engine deps via dummy [1,1] matmuls (walrus 2-wait
limit).
"""

import sys

sys.path.insert(0, "/opt/trn_rl_repo")

import numpy as np
import ml_dtypes

import concourse.bass as bass
import concourse.tile as tile
from concourse import bacc
from concourse import mybir
from concourse.bass_utils import run_bass_kernel_spmd
from concourse.masks import make_identity

BF = mybir.dt.bfloat16
F32 = mybir.dt.float32
F8 = mybir.dt.float8e4
WSCALE = 32.0    # host pre-scales projection weights into fp8e4m3 range
N = 2048
D = 1024
HD = 64
HPC = 4          # heads per core
DC = HPC * HD    # 256 head dims per core
NB = N // 128    # 16 row blocks
WMAX = N + 127   # max scratch row width

_CACHE = {}



def _build_nc():
    nc = bacc.Bacc()
    # fp8 value+residual pairs, DoubleRow pair-packed: [ver, kcp, p, 2, cols]
    xp = nc.dram_tensor("xp", [2, 4, 128, 2, N], F8, kind="ExternalInput")
    wqkvp = nc.dram_tensor("wqkvp", [2, 4, 128, 2, 3 * DC], F8,
                           kind="ExternalInput")
    eT = nc.dram_tensor("eT", [DC, N + 1], BF, kind="ExternalInput")
    wpT = nc.dram_tensor("wpT", [DC, D], BF, kind="ExternalInput")
    outp = nc.dram_tensor("outp", [N, D], BF, kind="ExternalOutput")
    scratch = nc.dram_tensor("scratch", [HPC * NB * 128 * WMAX], BF)
    BANDSTRIDE = NB * 128 * WMAX  # per-head stride in scratch elements

    from contextlib import ExitStack

    with tile.TileContext(nc) as tc, ExitStack() as ctx:
        pers = ctx.enter_context(tc.tile_pool(name="pers", bufs=1))
        psA = ctx.enter_context(tc.tile_pool(name="psA", bufs=6, space="PSUM"))
        psB = ctx.enter_context(tc.tile_pool(name="psB", bufs=2, space="PSUM"))
        ppool = ctx.enter_context(tc.tile_pool(name="ppool", bufs=3))
        scp = ctx.enter_context(tc.tile_pool(name="scp", bufs=3))
        oo = ctx.enter_context(tc.tile_pool(name="oo", bufs=3))
        llp = ctx.enter_context(tc.tile_pool(name="llp", bufs=3))

        # ---- persistent SBUF tensors ----
        xt = [[pers.tile([128, 2, N], F8, tag=f"xt{v}_{i}", name=f"xt{v}_{i}")
               for i in range(4)] for v in range(2)]
        wqkv = [[pers.tile([128, 2, 3 * DC], F8, tag=f"wq{v}_{i}",
                           name=f"wq{v}_{i}") for i in range(4)] for v in range(2)]
        wp = [pers.tile([128, D], BF, tag=f"wp{i}", name=f"wp{i}") for i in range(2)]
        et = [pers.tile([128, N + 1], BF, tag=f"et{i}", name=f"et{i}") for i in range(2)]
        qt = [pers.tile([128, N + 1], BF, tag=f"qt{i}", name=f"qt{i}") for i in range(2)]
        kt = [pers.tile([128, N], BF, tag=f"kt{i}", name=f"kt{i}") for i in range(2)]
        vaug = [pers.tile([128, HPC, HD + 1], BF, tag=f"va{i}", name=f"va{i}") for i in range(NB)]
        aot = [pers.tile([128, N], BF, tag=f"ao{i}", name=f"ao{i}") for i in range(2)]
        ident = pers.tile([128, 128], BF, tag="ident", name="ident")
        ones = pers.tile([1, 64], F32, tag="ones", name="ones")

        make_identity(nc, ident[:])
        nc.gpsimd.memset(ones[:], 1.0)
        for g in range(2):
            nc.gpsimd.memset(qt[g][:, 0:1], 0.0)

        for i in range(4):
            nc.sync.dma_start(wqkv[0][i][:], wqkvp[0, i, :, :, :])
            nc.sync.dma_start(xt[0][i][:], xp[0, i, :, :, :])
        for i in range(4):
            nc.sync.dma_start(wqkv[1][i][:], wqkvp[1, i, :, :, :])
            nc.sync.dma_start(xt[1][i][:], xp[1, i, :, :, :])
        for g in range(2):
            nc.sync.dma_start(wp[g][:], wpT[bass.ts(g, 128), :])
            nc.sync.dma_start(et[g][:], eT[bass.ts(g, 128), :])

        # ---- projections: error-compensated double-fp8 DoubleRow ----
        # q = x1.W1 + x1.W2 + x2.W1 (residual^2 term dropped), K=256/matmul
        DR = mybir.MatmulPerfMode.DoubleRow
        PASSES = [(0, 0), (0, 1), (1, 0)]

        def emit_proj_chunk(nchunk):
            for g in range(2):
                ps = psA.tile([128, 512], F32, tag="mm", name="mm")
                for pi, (xv, wv) in enumerate(PASSES):
                    for kc in range(4):
                        nc.tensor.matmul(
                            ps[:], wqkv[wv][kc][:, :, bass.ts(g, 128)],
                            xt[xv][kc][:, :, bass.ts(nchunk, 512)],
                            start=(pi == 0 and kc == 0),
                            stop=(pi == 2 and kc == 3), perf_mode=DR)
                nc.vector.tensor_scalar_mul(
                    qt[g][:, 1 + nchunk * 512:1 + (nchunk + 1) * 512],
                    ps[:], 1.0 / WSCALE)
                ps2 = psA.tile([128, 512], F32, tag="mm", name="mm")
                for pi, (xv, wv) in enumerate(PASSES):
                    for kc in range(4):
                        nc.tensor.matmul(
                            ps2[:],
                            wqkv[wv][kc][:, :, 256 + 128 * g:256 + 128 * (g + 1)],
                            xt[xv][kc][:, :, bass.ts(nchunk, 512)],
                            start=(pi == 0 and kc == 0),
                            stop=(pi == 2 and kc == 3), perf_mode=DR)
                nc.vector.tensor_scalar_mul(kt[g][:, bass.ts(nchunk, 512)],
                                            ps2[:], 1.0 / WSCALE)

        # ---- P bands (srel, diag-zero via padded e col) + skew bounce writes ----
        def emit_p_band(r0i):
            c_max = 128 * (r0i + 1)
            valid = c_max - 1          # data cols [0, valid); col valid = 0 (diag)
            W = c_max + 127            # scratch row stride
            m0 = N - valid             # first embedding index
            p4 = ppool.tile([128, HPC, WMAX], BF, tag="p4", name="p4")
            nc.gpsimd.memset(p4[:, :, c_max:W], -1e9)
            j = 0
            for h in range(HPC):
                g, ho = h // 2, 64 * (h % 2)
                for c0 in range(0, c_max, 512):
                    w = min(512, c_max - c0)
                    ps = psA.tile([128, 512], F32, tag="mm", name="mm")
                    nc.tensor.matmul(
                        ps[:, 0:w],
                        qt[g][ho:ho + 64, 128 * r0i:128 * r0i + 128],
                        et[g][ho:ho + 64, m0 + c0:m0 + c0 + w],
                        start=True, stop=True)
                    if j % 2 == 0:
                        nc.scalar.copy(p4[:, h, c0:c0 + w], ps[:, 0:w])
                    else:
                        nc.vector.tensor_copy(p4[:, h, c0:c0 + w], ps[:, 0:w])
                    j += 1
            base0 = r0i * 128 * WMAX
            if r0i >= 7:
                # split big band writes so transposes can slip in between
                for hp in range(2):
                    wr_ap = bass.AP(scratch, base0 + 2 * hp * BANDSTRIDE,
                                    [[W, 128], [BANDSTRIDE, 2], [1, W]])
                    nc.sync.dma_start(wr_ap, p4[:, 2 * hp:2 * hp + 2, 0:W])
            else:
                wr_ap = bass.AP(scratch, base0,
                                [[W, 128], [BANDSTRIDE, HPC], [1, W]])
                nc.sync.dma_start(wr_ap, p4[:, :, 0:W])

        # V-proj blocks interleaved with P bands: earlier band writes
        def emit_v_block(i):
            psv = psA.tile([128, HPC, HD], F32, tag="mm", name="mm")
            for pi, (xv, wv) in enumerate(PASSES):
                for kc in range(4):
                    nc.tensor.matmul(
                        psv[:], xt[xv][kc][:, :, bass.ts(i, 128)],
                        wqkv[wv][kc][:, :, 512:768],
                        start=(pi == 0 and kc == 0),
                        stop=(pi == 2 and kc == 3), perf_mode=DR)
            nc.gpsimd.memset(vaug[i][:, :, 64:65], 1.0)
            nc.vector.tensor_scalar_mul(vaug[i][:, :, 0:64], psv[:],
                                        1.0 / WSCALE)

        # progressive early phase: proj chunk 0 -> bands 0-3 + V 0-3 ->
        # proj chunk 1 -> bands 4-7 + V 4-7 -> ... so attention chunk 0
        # can start after only a quarter of the projection work
        for nchunk in range(4):
            emit_proj_chunk(nchunk)
            for b in range(4 * nchunk, 4 * nchunk + 4):
                if b % 2 == 1:
                    emit_v_block(b)
                    emit_p_band(b)
                else:
                    emit_p_band(b)
                    emit_v_block(b)

        # Funnel cross-engine deps into PE's observed clock so no real
        # matmul needs >2 sync waits (walrus MM wait-slot limit).
        srcs = [et[0], et[1], qt[0], qt[1], kt[0], kt[1], wp[0], wp[1]]
        for i, src in enumerate(srcs):
            if i % 2 == 0:
                ps_d = psA.tile([1, 1], F32, tag="mm", name="mm")
            else:
                ps_d = psB.tile([1, 1], F32, tag="av", name="av")
            nc.tensor.matmul(ps_d[0:1, 0:1], src[0:1, 1:2], src[0:1, 1:2],
                             start=True, stop=True)
        for i, src in enumerate([vaug[0], vaug[NB - 1]]):
            ps_d = psB.tile([1, 1], F32, tag="av", name="av")
            nc.tensor.matmul(ps_d[0:1, 0:1], src[0:1, 0, 0:1], src[0:1, 0, 0:1],
                             start=True, stop=True)

        # ---- attention, chunk by chunk (chunk c = n cols [512c, 512c+512)) ----
        groups = [(c, h) for c in range(4) for h in range(HPC)]

        def prologue(c, h):
            # SC alloc + skewed+transposed srel band reads + absent-region zeros
            SC = scp.tile([128, NB, 512], BF, tag="sc", name="sc")
            for r0i in range(4 * c, 4 * c + 4):
                c_max = 128 * (r0i + 1)
                W = c_max + 127
                base = (h * NB + r0i) * 128 * WMAX
                rd_ap = bass.AP(scratch, base + 127,
                                [[W - 1, 128], [1, c_max]])
                off = 128 * (r0i - 4 * c)
                nc.sync.dma_start(SC[:, 0:r0i + 1, off:off + 128], rd_ap,
                                  transpose=True)
            for kb in range(4 * c + 1, 4 * c + 4):
                tr = 128 * kb - 512 * c
                nc.gpsimd.memset(SC[:, kb, 0:tr], 0.0)
            return SC

        def emit_outproj(c):
            for r0i in range(4 * c, 4 * c + 4):
                o_sb = oo.tile([128, 1024], BF, tag="osb", name="osb")
                for nch in range(2):
                    ps = psA.tile([128, 512], F32, tag="mm", name="mm")
                    for dc in range(2):
                        nc.tensor.matmul(
                            ps[:], aot[dc][:, bass.ts(r0i, 128)],
                            wp[dc][:, bass.ts(nch, 512)],
                            start=(dc == 0), stop=(dc == 1))
                    if nch == 0:
                        nc.scalar.copy(o_sb[:, 0:512], ps[:])
                    else:
                        nc.vector.tensor_copy(o_sb[:, 512:1024], ps[:])
                nc.sync.dma_start(outp[bass.ts(r0i, 128), :], o_sb[:])

        LEAD = 2
        SCs = {i: prologue(*groups[i]) for i in range(LEAD)}

        def make_av_closure(SC, ps_av, h, nblk):
            emitted = [0]

            def emit_some(k):
                # funnel Pool-memset + psB-release sems into PE order once
                if emitted[0] == 0:
                    ps_d = psB.tile([1, 1], F32, tag="av", name="av")
                    nc.tensor.matmul(ps_d[0:1, 0:1], SC[0:1, 0, 0:1],
                                     SC[0:1, 0, 0:1], start=True, stop=True)
                while emitted[0] < min(k, nblk):
                    kb = emitted[0]
                    nc.tensor.matmul(
                        ps_av[0:65, :], vaug[kb][:, h, :], SC[:, kb, 0:512],
                        start=(kb == 0), stop=(kb == nblk - 1))
                    emitted[0] += 1

            return emit_some

        def emit_norm(ps_av, g, ho, c):
            # normalize: reciprocal + Pool partition-broadcast + multiply
            linv = llp.tile([1, 512], F32, tag="linv", name="linv")
            nc.vector.reciprocal(linv[:], ps_av[64:65, :])
            lb = llp.tile([64, 512], F32, tag="lb", name="lb")
            nc.gpsimd.partition_broadcast(lb[:], linv[:], channels=64)
            nc.vector.tensor_mul(
                aot[g][ho:ho + 64, 512 * c:512 * (c + 1)],
                ps_av[0:64, :], lb[:])

        prev = None  # (emit_some, nblk, ps_av, g, ho, c)
        for gi, (c, h) in enumerate(groups):
            g, ho = h // 2, 64 * (h % 2)
            nblk = 4 * c + 4
            SC = SCs.pop(gi)
            if gi + LEAD < len(groups):
                SCs[gi + LEAD] = prologue(*groups[gi + LEAD])
            ps_av = psB.tile([128, 512], F32, tag="av", name="av")
            own_av = make_av_closure(SC, ps_av, h, nblk)

            stt_n = 3
            for kb in range(nblk):
                tr = max(0, 128 * kb - 512 * c)
                w = 512 - tr
                use_stt = stt_n > 0 and (kb % stt_n == stt_n - 1)
                ps = psA.tile([128, 512], F32, tag="mm", name="mm")
                nc.tensor.matmul(
                    ps[:, 0:w],
                    kt[g][ho:ho + 64, 128 * kb:128 * kb + 128],
                    qt[g][ho:ho + 64, 1 + 512 * c + tr:1 + 512 * (c + 1)],
                    start=True, stop=(True if use_stt else False))
                if use_stt:
                    nc.vector.scalar_tensor_tensor(
                        SC[:, kb, tr:512], ps[:, 0:w], 1.0, SC[:, kb, tr:512],
                        mybir.AluOpType.mult, mybir.AluOpType.add)
                    nc.scalar.activation(
                        SC[:, kb, tr:512], SC[:, kb, tr:512],
                        mybir.ActivationFunctionType.Exp, scale=0.125)
                else:
                    nc.tensor.matmul(
                        ps[:, 0:w], ident[:], SC[:, kb, tr:512],
                        start=False, stop=True)
                    nc.scalar.activation(
                        SC[:, kb, tr:512], ps[:, 0:w],
                        mybir.ActivationFunctionType.Exp, scale=0.125)
                # interleave the PREVIOUS group's AV matmuls: their exps are
                # long done, so neither side ever waits on the other
                if prev is not None:
                    pav, pnblk = prev[0], prev[1]
                    pav((kb + 1) * pnblk // nblk)
                if gi == len(groups) - 1 and kb >= 2:
                    # last group: interleave its own AVs (lag 2) to cut the tail
                    own_av(kb - 1)

            if prev is not None:
                pav, pnblk, pps_av, pg, pho, pc = prev
                pav(pnblk)
                emit_norm(pps_av, pg, pho, pc)
                if pc != c:
                    # previous group finished chunk pc: emit its out-proj
                    emit_outproj(pc)
            prev = (own_av, nblk, ps_av, g, ho, c)
        pav, pnblk, pps_av, pg, pho, pc = prev
        pav(pnblk)
        emit_norm(pps_av, pg, pho, pc)
        emit_outproj(3)
    nc.compile()
    return nc


def kernel(x, Wq, Wk, Wv, Wp, bp, rel_embed):
    x = np.asarray(x, np.float32)
    bf = ml_dtypes.bfloat16
    if "nc" not in _CACHE:
        _CACHE["nc"] = _build_nc()
    nc = _CACHE["nc"]

    in_maps = []
    for core in range(8):
        b, hg = core // 4, core % 4
        c0 = hg * DC
        wq_s = np.asarray(Wq)[c0:c0 + DC, :].T
        wk_s = np.asarray(Wk)[c0:c0 + DC, :].T
        wv_s = np.asarray(Wv)[c0:c0 + DC, :].T
        e_s = np.asarray(rel_embed)[:, c0:c0 + DC].T        # [DC, N]
        e_pad = np.concatenate([e_s, np.zeros((DC, 1), e_s.dtype)], axis=1)
        f8 = ml_dtypes.float8_e4m3

        def pack(a):
            # [D, cols] -> [kcp, p, 2, cols]
            return np.ascontiguousarray(
                a.reshape(4, 2, 128, a.shape[1]).transpose(0, 2, 1, 3))

        xT_f = np.ascontiguousarray(x[b].T).astype(np.float32)
        x1 = xT_f.astype(f8)
        x2 = (xT_f - x1.astype(np.float32)).astype(f8)
        wcat = np.concatenate([wq_s, wk_s, wv_s], axis=1).astype(np.float32)
        wcat = wcat * WSCALE
        w1 = wcat.astype(f8)
        w2 = (wcat - w1.astype(np.float32)).astype(f8)
        in_maps.append({
            "xp": np.stack([pack(x1), pack(x2)]),
            "wqkvp": np.stack([pack(w1), pack(w2)]),
            "eT": np.ascontiguousarray(e_pad).astype(bf),
            "wpT": np.ascontiguousarray(np.asarray(Wp)[:, c0:c0 + DC].T).astype(bf),
        })
    kw = dict(_CACHE.get("run_kwargs") or {})
    r = run_bass_kernel_spmd(nc, in_maps, list(range(8)), **kw)
    _CACHE["last_result"] = r
    res = r.results
    out = np.zeros((2, N, D), np.float32)
    for core in range(8):
        out[core // 4] += np.asarray(res[core]["outp"], np.float32)
    out += np.asarray(bp, np.float32)
    return out


# revision 8
# speedup vs baseline: 1.0535x; 1.0208x over previous
"""Trainium2 Bass kernel for music-transformer relative attention.

Shapes (hardcoded): x [2, 2048, 1024], 16 heads x 64 dims, MAXLEN == N == 2048.
Sharding: 8 cores = 2 batches x 4 head-groups (4 heads each). Each core computes
its heads' attention and a partial output projection (bf16); host sums the 4
partials per batch in fp32 and adds the bias.

Per-core pipeline (transposed scores; no PE transposes, no A PSUM->SBUF copies):
  qt/kt [64*4, N(+1)] transposed layout; V in vaug [m, 4, 65] (ones col at 64
  yields softmax denominators through the AV matmul for free).
  srel: P[p, j] = q_{n0+p-1} . e_{m0+j} per 128-row band (col j==valid hits a
  zero-padded e column -> the diagonal zero falls out of the matmul), -1e9 tail,
  written contiguously to a DRAM scratch.
  The skewed read back uses dma_start_transpose on the strided skew AP: ONE DMA
  per (head, band) lands srel^T 128-blocks side-by-side in per-(head, n-chunk)
  slot tiles SC [128, 16, 512] (transpose cost rides the otherwise idle DMA
  track). Scores: PSUM = K.Q^T (wide 512-col matmuls) + I.T @ srelT (2 of 3
  slots) or a DVE scalar_tensor_tensor add (every 3rd slot, balancing PE/DVE);
  exp on ACT writes A^T in place over srelT. The causal mask is the baked -1e9
  tail (exp -> exact 0); absent sub-blocks are Pool-memset to 0 so AV runs full
  512 wide. AV: vaug^T @ A^T accumulates out^T + row sums.
  Normalize: DVE reciprocal -> Pool partition_broadcast -> DVE multiply (no PE
  and no ACT in the chain). Out-proj from the transposed layout.

Schedule: progressive early phase (proj chunk k -> P bands/V blocks 4k..4k+3)
so attention chunk 0 starts after a quarter of the projection work; per-group
prologues (band transposes + slot memsets) emitted 2 groups ahead and
dispatched from SP so semaphore waits never block the ACT sequencer; each
group's AV matmuls are emitted interleaved into the NEXT group's score stream
(their exps are long done -> no exp->AV stall, scores never wait on AV); the
out-projection of chunk c is deferred past the first group of chunk c+1.
PE matmuls funnel cross-engine deps via dummy [1,1] matmuls (walrus 2-wait
limit).
"""

import sys

sys.path.insert(0, "/opt/trn_rl_repo")

import numpy as np
import ml_dtypes

import concourse.bass as bass
import concourse.tile as tile
from concourse import bacc
from concourse import mybir
from concourse.bass_utils import run_bass_kernel_spmd
from concourse.masks import make_identity

BF = mybir.dt.bfloat16
F32 = mybir.dt.float32
F8 = mybir.dt.float8e4
WSCALE = 32.0    # host pre-scales projection weights into fp8e4m3 range
N = 2048
D = 1024
HD = 64
HPC = 4          # heads per core
DC = HPC * HD    # 256 head dims per core
NB = N // 128    # 16 row blocks
WMAX = N + 127   # max scratch row width

_CACHE = {}



def _build_nc():
    nc = bacc.Bacc()
    # fp8 value+residual pairs, DoubleRow pair-packed: [ver, kcp, p, 2, cols]
    xp = nc.dram_tensor("xp", [2, 4, 128, 2, N], F8, kind="ExternalInput")
    wqkvp = nc.dram_tensor("wqkvp", [2, 4, 128, 2, 3 * DC], F8,
                           kind="ExternalInput")
    eT = nc.dram_tensor("eT", [DC, N + 1], BF, kind="ExternalInput")
    wpT = nc.dram_tensor("wpT", [DC, D], BF, kind="ExternalInput")
    outp = nc.dram_tensor("outp", [N, D], BF, kind="ExternalOutput")
    scratch = nc.dram_tensor("scratch", [HPC * NB * 128 * WMAX], BF)
    BANDSTRIDE = NB * 128 * WMAX  # per-head stride in scratch elements

    from contextlib import ExitStack

    with tile.TileContext(nc) as tc, ExitStack() as ctx:
        pers = ctx.enter_context(tc.tile_pool(name="pers", bufs=1))
        psA = ctx.enter_context(tc.tile_pool(name="psA", bufs=6, space="PSUM"))
        psB = ctx.enter_context(tc.tile_pool(name="psB", bufs=2, space="PSUM"))
        ppool = ctx.enter_context(tc.tile_pool(name="ppool", bufs=3))
        scp = ctx.enter_context(tc.tile_pool(name="scp", bufs=3))
        oo = ctx.enter_context(tc.tile_pool(name="oo", bufs=3))
        llp = ctx.enter_context(tc.tile_pool(name="llp", bufs=3))

        # ---- persistent SBUF tensors ----
        xt = [[pers.tile([128, 2, N], F8, tag=f"xt{v}_{i}", name=f"xt{v}_{i}")
               for i in range(4)] for v in range(2)]
        wqkv = [[pers.tile([128, 2, 3 * DC], F8, tag=f"wq{v}_{i}",
                           name=f"wq{v}_{i}") for i in range(4)] for v in range(2)]
        wp = [pers.tile([128, D], BF, tag=f"wp{i}", name=f"wp{i}") for i in range(2)]
        et = [pers.tile([128, N + 1], BF, tag=f"et{i}", name=f"et{i}") for i in range(2)]
        qt = [pers.tile([128, N + 1], BF, tag=f"qt{i}", name=f"qt{i}") for i in range(2)]
        kt = [pers.tile([128, N], BF, tag=f"kt{i}", name=f"kt{i}") for i in range(2)]
        vaug = [pers.tile([128, HPC, HD + 1], BF, tag=f"va{i}", name=f"va{i}") for i in range(NB)]
        aot = [pers.tile([128, N], BF, tag=f"ao{i}", name=f"ao{i}") for i in range(2)]
        ident = pers.tile([128, 128], BF, tag="ident", name="ident")
        ones = pers.tile([1, 64], F32, tag="ones", name="ones")

        make_identity(nc, ident[:])
        nc.gpsimd.memset(ones[:], 1.0)
        for g in range(2):
            nc.gpsimd.memset(qt[g][:, 0:1], 0.0)

        for i in range(4):
            nc.sync.dma_start(wqkv[0][i][:], wqkvp[0, i, :, :, :])
            nc.sync.dma_start(xt[0][i][:], xp[0, i, :, :, :])
        for i in range(4):
            nc.sync.dma_start(wqkv[1][i][:], wqkvp[1, i, :, :, :])
            nc.sync.dma_start(xt[1][i][:], xp[1, i, :, :, :])
        for g in range(2):
            nc.sync.dma_start(wp[g][:], wpT[bass.ts(g, 128), :])
            nc.sync.dma_start(et[g][:], eT[bass.ts(g, 128), :])

        # ---- projections: error-compensated double-fp8 DoubleRow ----
        # q = x1.W1 + x1.W2 + x2.W1 (residual^2 term dropped), K=256/matmul
        DR = mybir.MatmulPerfMode.DoubleRow
        PASSES = [(0, 0), (0, 1), (1, 0)]

        def emit_proj_chunk(nchunk):
            for g in range(2):
                ps = psA.tile([128, 512], F32, tag="mm", name="mm")
                for pi, (xv, wv) in enumerate(PASSES):
                    for kc in range(4):
                        nc.tensor.matmul(
                            ps[:], wqkv[wv][kc][:, :, bass.ts(g, 128)],
                            xt[xv][kc][:, :, bass.ts(nchunk, 512)],
                            start=(pi == 0 and kc == 0),
                            stop=(pi == 2 and kc == 3), perf_mode=DR)
                nc.vector.tensor_scalar_mul(
                    qt[g][:, 1 + nchunk * 512:1 + (nchunk + 1) * 512],
                    ps[:], 1.0 / WSCALE)
                ps2 = psA.tile([128, 512], F32, tag="mm", name="mm")
                for pi, (xv, wv) in enumerate(PASSES):
                    for kc in range(4):
                        nc.tensor.matmul(
                            ps2[:],
                            wqkv[wv][kc][:, :, 256 + 128 * g:256 + 128 * (g + 1)],
                            xt[xv][kc][:, :, bass.ts(nchunk, 512)],
                            start=(pi == 0 and kc == 0),
                            stop=(pi == 2 and kc == 3), perf_mode=DR)
                nc.vector.tensor_scalar_mul(kt[g][:, bass.ts(nchunk, 512)],
                                            ps2[:], 1.0 / WSCALE)

        # ---- P bands (srel, diag-zero via padded e col) + skew bounce writes ----
        def emit_p_band(r0i):
            c_max = 128 * (r0i + 1)
            valid = c_max - 1          # data cols [0, valid); col valid = 0 (diag)
            W = c_max + 127            # scratch row stride
            m0 = N - valid             # first embedding index
            p4 = ppool.tile([128, HPC, WMAX], BF, tag="p4", name="p4")
            nc.gpsimd.memset(p4[:, :, c_max:W], -1e9)
            j = 0
            for h in range(HPC):
                g, ho = h // 2, 64 * (h % 2)
                for c0 in range(0, c_max, 512):
                    w = min(512, c_max - c0)
                    ps = psA.tile([128, 512], F32, tag="mm", name="mm")
                    nc.tensor.matmul(
                        ps[:, 0:w],
                        qt[g][ho:ho + 64, 128 * r0i:128 * r0i + 128],
                        et[g][ho:ho + 64, m0 + c0:m0 + c0 + w],
                        start=True, stop=True)
                    if j % 2 == 0:
                        nc.scalar.copy(p4[:, h, c0:c0 + w], ps[:, 0:w])
                    else:
                        nc.vector.tensor_copy(p4[:, h, c0:c0 + w], ps[:, 0:w])
                    j += 1
            base0 = r0i * 128 * WMAX
            if r0i >= 7:
                # split big band writes so transposes can slip in between
                for hp in range(2):
                    wr_ap = bass.AP(scratch, base0 + 2 * hp * BANDSTRIDE,
                                    [[W, 128], [BANDSTRIDE, 2], [1, W]])
                    nc.sync.dma_start(wr_ap, p4[:, 2 * hp:2 * hp + 2, 0:W])
            else:
                wr_ap = bass.AP(scratch, base0,
                                [[W, 128], [BANDSTRIDE, HPC], [1, W]])
                nc.sync.dma_start(wr_ap, p4[:, :, 0:W])

        # V-proj blocks interleaved with P bands: earlier band writes
        def emit_v_block(i):
            psv = psA.tile([128, HPC, HD], F32, tag="mm", name="mm")
            for pi, (xv, wv) in enumerate(PASSES):
                for kc in range(4):
                    nc.tensor.matmul(
                        psv[:], xt[xv][kc][:, :, bass.ts(i, 128)],
                        wqkv[wv][kc][:, :, 512:768],
                        start=(pi == 0 and kc == 0),
                        stop=(pi == 2 and kc == 3), perf_mode=DR)
            nc.gpsimd.memset(vaug[i][:, :, 64:65], 1.0)
            nc.vector.tensor_scalar_mul(vaug[i][:, :, 0:64], psv[:],
                                        1.0 / WSCALE)

        # progressive early phase: proj chunk 0 -> bands 0-3 + V 0-3 ->
        # proj chunk 1 -> bands 4-7 + V 4-7 -> ... so attention chunk 0
        # can start after only a quarter of the projection work
        for nchunk in range(4):
            emit_proj_chunk(nchunk)
            for b in range(4 * nchunk, 4 * nchunk + 4):
                if b % 2 == 1:
                    emit_v_block(b)
                    emit_p_band(b)
                else:
                    emit_p_band(b)
                    emit_v_block(b)

        # Funnel cross-engine deps into PE's observed clock so no real
        # matmul needs >2 sync waits (walrus MM wait-slot limit).
        srcs = [et[0], et[1], qt[0], qt[1], kt[0], kt[1], wp[0], wp[1]]
        for i, src in enumerate(srcs):
            if i % 2 == 0:
                ps_d = psA.tile([1, 1], F32, tag="mm", name="mm")
            else:
                ps_d = psB.tile([1, 1], F32, tag="av", name="av")
            nc.tensor.matmul(ps_d[0:1, 0:1], src[0:1, 1:2], src[0:1, 1:2],
                             start=True, stop=True)
        for i, src in enumerate([vaug[0], vaug[NB - 1]]):
            ps_d = psB.tile([1, 1], F32, tag="av", name="av")
            nc.tensor.matmul(ps_d[0:1, 0:1], src[0:1, 0, 0:1], src[0:1, 0, 0:1],
                             start=True, stop=True)

        # ---- attention, chunk by chunk (chunk c = n cols [512c, 512c+512)) ----
        groups = [(c, h) for c in range(4) for h in range(HPC)]

        def prologue(c, h):
            # SC alloc + skewed+transposed srel band reads + absent-region zeros
            SC = scp.tile([128, NB, 512], BF, tag="sc", name="sc")
            for r0i in range(4 * c, 4 * c + 4):
                c_max = 128 * (r0i + 1)
                W = c_max + 127
                base = (h * NB + r0i) * 128 * WMAX
                rd_ap = bass.AP(scratch, base + 127,
                                [[W - 1, 128], [1, c_max]])
                off = 128 * (r0i - 4 * c)
                nc.sync.dma_start(SC[:, 0:r0i + 1, off:off + 128], rd_ap,
                                  transpose=True)
            for kb in range(4 * c + 1, 4 * c + 4):
                tr = 128 * kb - 512 * c
                nc.gpsimd.memset(SC[:, kb, 0:tr], 0.0)
            return SC

        def emit_outproj(c):
            for r0i in range(4 * c, 4 * c + 4):
                o_sb = oo.tile([128, 1024], BF, tag="osb", name="osb")
                for nch in range(2):
                    ps = psA.tile([128, 512], F32, tag="mm", name="mm")
                    for dc in range(2):
                        nc.tensor.matmul(
                            ps[:], aot[dc][:, bass.ts(r0i, 128)],
                            wp[dc][:, bass.ts(nch, 512)],
                            start=(dc == 0), stop=(dc == 1))
                    if nch == 1:
                        nc.scalar.copy(o_sb[:, 512:1024], ps[:])
                    else:
                        nc.vector.tensor_copy(o_sb[:, 0:512], ps[:])
                nc.sync.dma_start(outp[bass.ts(r0i, 128), :], o_sb[:])

        LEAD = 2
        SCs = {i: prologue(*groups[i]) for i in range(LEAD)}

        def make_av_closure(SC, ps_av, h, nblk):
            emitted = [0]

            def emit_some(k):
                # funnel Pool-memset + psB-release sems into PE order once
                if emitted[0] == 0:
                    ps_d = psB.tile([1, 1], F32, tag="av", name="av")
                    nc.tensor.matmul(ps_d[0:1, 0:1], SC[0:1, 0, 0:1],
                                     SC[0:1, 0, 0:1], start=True, stop=True)
                while emitted[0] < min(k, nblk):
                    kb = emitted[0]
                    nc.tensor.matmul(
                        ps_av[0:65, :], vaug[kb][:, h, :], SC[:, kb, 0:512],
                        start=(kb == 0), stop=(kb == nblk - 1))
                    emitted[0] += 1

            return emit_some

        def emit_norm(ps_av, g, ho, c):
            # normalize: reciprocal + Pool partition-broadcast + multiply
            linv = llp.tile([1, 512], F32, tag="linv", name="linv")
            nc.vector.reciprocal(linv[:], ps_av[64:65, :])
            lb = llp.tile([64, 512], F32, tag="lb", name="lb")
            nc.gpsimd.partition_broadcast(lb[:], linv[:], channels=64)
            nc.vector.tensor_mul(
                aot[g][ho:ho + 64, 512 * c:512 * (c + 1)],
                ps_av[0:64, :], lb[:])

        prev = None  # (emit_some, nblk, ps_av, g, ho, c)
        for gi, (c, h) in enumerate(groups):
            g, ho = h // 2, 64 * (h % 2)
            nblk = 4 * c + 4
            SC = SCs.pop(gi)
            if gi + LEAD < len(groups):
                SCs[gi + LEAD] = prologue(*groups[gi + LEAD])
            ps_av = psB.tile([128, 512], F32, tag="av", name="av")
            own_av = make_av_closure(SC, ps_av, h, nblk)

            stt_n = 3
            for kb in range(nblk):
                tr = max(0, 128 * kb - 512 * c)
                w = 512 - tr
                use_stt = stt_n > 0 and (kb % stt_n == stt_n - 1)
                ps = psA.tile([128, 512], F32, tag="mm", name="mm")
                nc.tensor.matmul(
                    ps[:, 0:w],
                    kt[g][ho:ho + 64, 128 * kb:128 * kb + 128],
                    qt[g][ho:ho + 64, 1 + 512 * c + tr:1 + 512 * (c + 1)],
                    start=True, stop=(True if use_stt else False))
                if use_stt:
                    nc.vector.scalar_tensor_tensor(
                        SC[:, kb, tr:512], ps[:, 0:w], 1.0, SC[:, kb, tr:512],
                        mybir.AluOpType.mult, mybir.AluOpType.add)
                    nc.scalar.activation(
                        SC[:, kb, tr:512], SC[:, kb, tr:512],
                        mybir.ActivationFunctionType.Exp, scale=0.125)
                else:
                    nc.tensor.matmul(
                        ps[:, 0:w], ident[:], SC[:, kb, tr:512],
                        start=False, stop=True)
                    nc.scalar.activation(
                        SC[:, kb, tr:512], ps[:, 0:w],
                        mybir.ActivationFunctionType.Exp, scale=0.125)
                # interleave the PREVIOUS group's AV matmuls: their exps are
                # long done, so neither side ever waits on the other
                if prev is not None:
                    pav, pnblk = prev[0], prev[1]
                    pav((kb + 1) * pnblk // nblk)
                if gi == len(groups) - 1 and kb >= 2:
                    # last group: interleave its own AVs (lag 2) to cut the tail
                    own_av(kb - 1)

            if prev is not None:
                pav, pnblk, pps_av, pg, pho, pc = prev
                pav(pnblk)
                emit_norm(pps_av, pg, pho, pc)
                if pc != c:
                    # previous group finished chunk pc: emit its out-proj
                    emit_outproj(pc)
            prev = (own_av, nblk, ps_av, g, ho, c)
        pav, pnblk, pps_av, pg, pho, pc = prev
        pav(pnblk)
        emit_norm(pps_av, pg, pho, pc)
        emit_outproj(3)
    nc.compile()
    return nc


def kernel(x, Wq, Wk, Wv, Wp, bp, rel_embed):
    x = np.asarray(x, np.float32)
    bf = ml_dtypes.bfloat16
    if "nc" not in _CACHE:
        _CACHE["nc"] = _build_nc()
    nc = _CACHE["nc"]

    in_maps = []
    for core in range(8):
        b, hg = core // 4, core % 4
        c0 = hg * DC
        wq_s = np.asarray(Wq)[c0:c0 + DC, :].T
        wk_s = np.asarray(Wk)[c0:c0 + DC, :].T
        wv_s = np.asarray(Wv)[c0:c0 + DC, :].T
        e_s = np.asarray(rel_embed)[:, c0:c0 + DC].T        # [DC, N]
        e_pad = np.concatenate([e_s, np.zeros((DC, 1), e_s.dtype)], axis=1)
        f8 = ml_dtypes.float8_e4m3

        def pack(a):
            # [D, cols] -> [kcp, p, 2, cols]
            return np.ascontiguousarray(
                a.reshape(4, 2, 128, a.shape[1]).transpose(0, 2, 1, 3))

        xT_f = np.ascontiguousarray(x[b].T).astype(np.float32)
        x1 = xT_f.astype(f8)
        x2 = (xT_f - x1.astype(np.float32)).astype(f8)
        wcat = np.concatenate([wq_s, wk_s, wv_s], axis=1).astype(np.float32)
        wcat = wcat * WSCALE
        w1 = wcat.astype(f8)
        w2 = (wcat - w1.astype(np.float32)).astype(f8)
        in_maps.append({
            "xp": np.stack([pack(x1), pack(x2)]),
            "wqkvp": np.stack([pack(w1), pack(w2)]),
            "eT": np.ascontiguousarray(e_pad).astype(bf),
            "wpT": np.ascontiguousarray(np.asarray(Wp)[:, c0:c0 + DC].T).astype(bf),
        })
    kw = dict(_CACHE.get("run_kwargs") or {})
    r = run_bass_kernel_spmd(nc, in_maps, list(range(8)), **kw)
    _CACHE["last_result"] = r
    res = r.results
    out = np.zeros((2, N, D), np.float32)
    for core in range(8):
        out[core // 4] += np.asarray(res[core]["outp"], np.float32)
    out += np.asarray(bp, np.float32)
    return out


# revision 9
# speedup vs baseline: 1.0577x; 1.0040x over previous
"""Trainium2 Bass kernel for music-transformer relative attention.

Shapes (hardcoded): x [2, 2048, 1024], 16 heads x 64 dims, MAXLEN == N == 2048.
Sharding: 8 cores = 2 batches x 4 head-groups (4 heads each). Each core computes
its heads' attention and a partial output projection (bf16); host sums the 4
partials per batch in fp32 and adds the bias.

Per-core pipeline (transposed scores; no PE transposes, no A PSUM->SBUF copies):
  qt/kt [64*4, N(+1)] transposed layout; V in vaug [m, 4, 65] (ones col at 64
  yields softmax denominators through the AV matmul for free).
  srel: P[p, j] = q_{n0+p-1} . e_{m0+j} per 128-row band (col j==valid hits a
  zero-padded e column -> the diagonal zero falls out of the matmul), -1e9 tail,
  written contiguously to a DRAM scratch.
  The skewed read back uses dma_start_transpose on the strided skew AP: ONE DMA
  per (head, band) lands srel^T 128-blocks side-by-side in per-(head, n-chunk)
  slot tiles SC [128, 16, 512] (transpose cost rides the otherwise idle DMA
  track). Scores: PSUM = K.Q^T (wide 512-col matmuls) + I.T @ srelT (2 of 3
  slots) or a DVE scalar_tensor_tensor add (every 3rd slot, balancing PE/DVE);
  exp on ACT writes A^T in place over srelT. The causal mask is the baked -1e9
  tail (exp -> exact 0); absent sub-blocks are Pool-memset to 0 so AV runs full
  512 wide. AV: vaug^T @ A^T accumulates out^T + row sums.
  Normalize: DVE reciprocal -> Pool partition_broadcast -> DVE multiply (no PE
  and no ACT in the chain). Out-proj from the transposed layout.

Schedule: progressive early phase (proj chunk k -> P bands/V blocks 4k..4k+3)
so attention chunk 0 starts after a quarter of the projection work; per-group
prologues (band transposes + slot memsets) emitted 2 groups ahead and
dispatched from SP so semaphore waits never block the ACT sequencer; each
group's AV matmuls are emitted interleaved into the NEXT group's score stream
(their exps are long done -> no exp->AV stall, scores never wait on AV); the
out-projection of chunk c is deferred past the first group of chunk c+1.
PE matmuls funnel cross-engine deps via dummy [1,1] matmuls (walrus 2-wait
limit).
"""

import sys

sys.path.insert(0, "/opt/trn_rl_repo")

import numpy as np
import ml_dtypes

import concourse.bass as bass
import concourse.tile as tile
from concourse import bacc
from concourse import mybir
from concourse.bass_utils import run_bass_kernel_spmd
from concourse.masks import make_identity

BF = mybir.dt.bfloat16
F32 = mybir.dt.float32
F8 = mybir.dt.float8e4
WSCALE = 32.0    # host pre-scales projection weights into fp8e4m3 range
N = 2048
D = 1024
HD = 64
HPC = 4          # heads per core
DC = HPC * HD    # 256 head dims per core
NB = N // 128    # 16 row blocks
WMAX = N + 127   # max scratch row width

_CACHE = {}



def _build_nc():
    nc = bacc.Bacc()
    # fp8 value+residual pairs, DoubleRow pair-packed: [ver, kcp, p, 2, cols]
    xp = nc.dram_tensor("xp", [2, 4, 128, 2, N], F8, kind="ExternalInput")
    wqkvp = nc.dram_tensor("wqkvp", [2, 4, 128, 2, 3 * DC], F8,
                           kind="ExternalInput")
    eT = nc.dram_tensor("eT", [DC, N + 1], BF, kind="ExternalInput")
    wpT = nc.dram_tensor("wpT", [DC, D], BF, kind="ExternalInput")
    outp = nc.dram_tensor("outp", [N, D], BF, kind="ExternalOutput")
    scratch = nc.dram_tensor("scratch", [HPC * NB * 128 * WMAX], BF)
    BANDSTRIDE = NB * 128 * WMAX  # per-head stride in scratch elements

    from contextlib import ExitStack

    with tile.TileContext(nc) as tc, ExitStack() as ctx:
        pers = ctx.enter_context(tc.tile_pool(name="pers", bufs=1))
        psA = ctx.enter_context(tc.tile_pool(name="psA", bufs=6, space="PSUM"))
        psB = ctx.enter_context(tc.tile_pool(name="psB", bufs=2, space="PSUM"))
        ppool = ctx.enter_context(tc.tile_pool(name="ppool", bufs=3))
        scp = ctx.enter_context(tc.tile_pool(name="scp", bufs=3))
        oo = ctx.enter_context(tc.tile_pool(name="oo", bufs=3))
        llp = ctx.enter_context(tc.tile_pool(name="llp", bufs=3))

        # ---- persistent SBUF tensors ----
        xt = [[pers.tile([128, 2, N], F8, tag=f"xt{v}_{i}", name=f"xt{v}_{i}")
               for i in range(4)] for v in range(2)]
        wqkv = [[pers.tile([128, 2, 3 * DC], F8, tag=f"wq{v}_{i}",
                           name=f"wq{v}_{i}") for i in range(4)] for v in range(2)]
        wp = [pers.tile([128, D], BF, tag=f"wp{i}", name=f"wp{i}") for i in range(2)]
        et = [pers.tile([128, N + 1], BF, tag=f"et{i}", name=f"et{i}") for i in range(2)]
        qt = [pers.tile([128, N + 1], BF, tag=f"qt{i}", name=f"qt{i}") for i in range(2)]
        kt = [pers.tile([128, N], BF, tag=f"kt{i}", name=f"kt{i}") for i in range(2)]
        vaug = [pers.tile([128, HPC, HD + 1], BF, tag=f"va{i}", name=f"va{i}") for i in range(NB)]
        aot = [pers.tile([128, N], BF, tag=f"ao{i}", name=f"ao{i}") for i in range(2)]
        ident = pers.tile([128, 128], BF, tag="ident", name="ident")
        ones = pers.tile([1, 64], F32, tag="ones", name="ones")

        make_identity(nc, ident[:])
        nc.gpsimd.memset(ones[:], 1.0)
        for g in range(2):
            nc.gpsimd.memset(qt[g][:, 0:1], 0.0)

        for i in range(4):
            nc.sync.dma_start(wqkv[0][i][:], wqkvp[0, i, :, :, :])
            nc.sync.dma_start(xt[0][i][:], xp[0, i, :, :, :])
        for i in range(4):
            nc.sync.dma_start(wqkv[1][i][:], wqkvp[1, i, :, :, :])
            nc.sync.dma_start(xt[1][i][:], xp[1, i, :, :, :])
        for g in range(2):
            nc.sync.dma_start(wp[g][:], wpT[bass.ts(g, 128), :])
            nc.sync.dma_start(et[g][:], eT[bass.ts(g, 128), :])

        # ---- projections: error-compensated double-fp8 DoubleRow ----
        # q = x1.W1 + x1.W2 + x2.W1 (residual^2 term dropped), K=256/matmul
        DR = mybir.MatmulPerfMode.DoubleRow
        PASSES = [(0, 0), (0, 1), (1, 0)]

        def emit_proj_chunk(nchunk):
            for g in range(2):
                ps = psA.tile([128, 512], F32, tag="mm", name="mm")
                for pi, (xv, wv) in enumerate(PASSES):
                    for kc in range(4):
                        nc.tensor.matmul(
                            ps[:], wqkv[wv][kc][:, :, bass.ts(g, 128)],
                            xt[xv][kc][:, :, bass.ts(nchunk, 512)],
                            start=(pi == 0 and kc == 0),
                            stop=(pi == 2 and kc == 3), perf_mode=DR)
                nc.vector.tensor_scalar_mul(
                    qt[g][:, 1 + nchunk * 512:1 + (nchunk + 1) * 512],
                    ps[:], 1.0 / WSCALE)
                ps2 = psA.tile([128, 512], F32, tag="mm", name="mm")
                for pi, (xv, wv) in enumerate(PASSES):
                    for kc in range(4):
                        nc.tensor.matmul(
                            ps2[:],
                            wqkv[wv][kc][:, :, 256 + 128 * g:256 + 128 * (g + 1)],
                            xt[xv][kc][:, :, bass.ts(nchunk, 512)],
                            start=(pi == 0 and kc == 0),
                            stop=(pi == 2 and kc == 3), perf_mode=DR)
                nc.vector.tensor_scalar_mul(kt[g][:, bass.ts(nchunk, 512)],
                                            ps2[:], 1.0 / WSCALE)

        # ---- P bands (srel, diag-zero via padded e col) + skew bounce writes ----
        def emit_p_band(r0i):
            c_max = 128 * (r0i + 1)
            valid = c_max - 1          # data cols [0, valid); col valid = 0 (diag)
            W = c_max + 127            # scratch row stride
            m0 = N - valid             # first embedding index
            p4 = ppool.tile([128, HPC, WMAX], BF, tag="p4", name="p4")
            nc.gpsimd.memset(p4[:, :, c_max:W], -1e9)
            j = 0
            for h in range(HPC):
                g, ho = h // 2, 64 * (h % 2)
                for c0 in range(0, c_max, 512):
                    w = min(512, c_max - c0)
                    ps = psA.tile([128, 512], F32, tag="mm", name="mm")
                    nc.tensor.matmul(
                        ps[:, 0:w],
                        qt[g][ho:ho + 64, 128 * r0i:128 * r0i + 128],
                        et[g][ho:ho + 64, m0 + c0:m0 + c0 + w],
                        start=True, stop=True)
                    if j % 2 == 0:
                        nc.scalar.copy(p4[:, h, c0:c0 + w], ps[:, 0:w])
                    else:
                        nc.vector.tensor_copy(p4[:, h, c0:c0 + w], ps[:, 0:w])
                    j += 1
            base0 = r0i * 128 * WMAX
            if r0i >= 7:
                # split big band writes so transposes can slip in between
                for hp in range(2):
                    wr_ap = bass.AP(scratch, base0 + 2 * hp * BANDSTRIDE,
                                    [[W, 128], [BANDSTRIDE, 2], [1, W]])
                    nc.sync.dma_start(wr_ap, p4[:, 2 * hp:2 * hp + 2, 0:W])
            else:
                wr_ap = bass.AP(scratch, base0,
                                [[W, 128], [BANDSTRIDE, HPC], [1, W]])
                nc.sync.dma_start(wr_ap, p4[:, :, 0:W])

        # V-proj blocks interleaved with P bands: earlier band writes
        def emit_v_block(i):
            psv = psB.tile([128, HPC, HD], F32, tag="av", name="av")
            for pi, (xv, wv) in enumerate(PASSES):
                for kc in range(4):
                    nc.tensor.matmul(
                        psv[:], xt[xv][kc][:, :, bass.ts(i, 128)],
                        wqkv[wv][kc][:, :, 512:768],
                        start=(pi == 0 and kc == 0),
                        stop=(pi == 2 and kc == 3), perf_mode=DR)
            nc.gpsimd.memset(vaug[i][:, :, 64:65], 1.0)
            nc.vector.tensor_scalar_mul(vaug[i][:, :, 0:64], psv[:],
                                        1.0 / WSCALE)

        # progressive early phase: proj chunk 0 -> bands 0-3 + V 0-3 ->
        # proj chunk 1 -> bands 4-7 + V 4-7 -> ... so attention chunk 0
        # can start after only a quarter of the projection work
        for nchunk in range(4):
            emit_proj_chunk(nchunk)
            for b in range(4 * nchunk, 4 * nchunk + 4):
                if b % 2 == 1:
                    emit_v_block(b)
                    emit_p_band(b)
                else:
                    emit_p_band(b)
                    emit_v_block(b)

        # Funnel cross-engine deps into PE's observed clock so no real
        # matmul needs >2 sync waits (walrus MM wait-slot limit).
        srcs = [et[0], et[1], qt[0], qt[1], kt[0], kt[1], wp[0], wp[1]]
        for i, src in enumerate(srcs):
            if i % 2 == 0:
                ps_d = psA.tile([1, 1], F32, tag="mm", name="mm")
            else:
                ps_d = psB.tile([1, 1], F32, tag="av", name="av")
            nc.tensor.matmul(ps_d[0:1, 0:1], src[0:1, 1:2], src[0:1, 1:2],
                             start=True, stop=True)
        for i, src in enumerate([vaug[0], vaug[NB - 1]]):
            ps_d = psB.tile([1, 1], F32, tag="av", name="av")
            nc.tensor.matmul(ps_d[0:1, 0:1], src[0:1, 0, 0:1], src[0:1, 0, 0:1],
                             start=True, stop=True)

        # ---- attention, chunk by chunk (chunk c = n cols [512c, 512c+512)) ----
        groups = [(c, h) for c in range(4) for h in range(HPC)]

        def prologue(c, h):
            # SC alloc + skewed+transposed srel band reads + absent-region zeros
            SC = scp.tile([128, NB, 512], BF, tag="sc", name="sc")
            for r0i in range(4 * c, 4 * c + 4):
                c_max = 128 * (r0i + 1)
                W = c_max + 127
                base = (h * NB + r0i) * 128 * WMAX
                rd_ap = bass.AP(scratch, base + 127,
                                [[W - 1, 128], [1, c_max]])
                off = 128 * (r0i - 4 * c)
                nc.sync.dma_start(SC[:, 0:r0i + 1, off:off + 128], rd_ap,
                                  transpose=True)
            for kb in range(4 * c + 1, 4 * c + 4):
                tr = 128 * kb - 512 * c
                nc.gpsimd.memset(SC[:, kb, 0:tr], 0.0)
            return SC

        def emit_outproj(c):
            for r0i in range(4 * c, 4 * c + 4):
                o_sb = oo.tile([128, 1024], BF, tag="osb", name="osb")
                for nch in range(2):
                    ps = psA.tile([128, 512], F32, tag="mm", name="mm")
                    for dc in range(2):
                        nc.tensor.matmul(
                            ps[:], aot[dc][:, bass.ts(r0i, 128)],
                            wp[dc][:, bass.ts(nch, 512)],
                            start=(dc == 0), stop=(dc == 1))
                    if nch == 1:
                        nc.scalar.copy(o_sb[:, 512:1024], ps[:])
                    else:
                        nc.vector.tensor_copy(o_sb[:, 0:512], ps[:])
                nc.sync.dma_start(outp[bass.ts(r0i, 128), :], o_sb[:])

        LEAD = 2
        SCs = {i: prologue(*groups[i]) for i in range(LEAD)}

        def make_av_closure(SC, ps_av, h, nblk):
            emitted = [0]

            def emit_some(k):
                # funnel Pool-memset + psB-release sems into PE order once
                if emitted[0] == 0:
                    ps_d = psB.tile([1, 1], F32, tag="av", name="av")
                    nc.tensor.matmul(ps_d[0:1, 0:1], SC[0:1, 0, 0:1],
                                     SC[0:1, 0, 0:1], start=True, stop=True)
                while emitted[0] < min(k, nblk):
                    kb = emitted[0]
                    nc.tensor.matmul(
                        ps_av[0:65, :], vaug[kb][:, h, :], SC[:, kb, 0:512],
                        start=(kb == 0), stop=(kb == nblk - 1))
                    emitted[0] += 1

            return emit_some

        def emit_norm(ps_av, g, ho, c):
            # normalize: reciprocal + Pool partition-broadcast + multiply
            linv = llp.tile([1, 512], F32, tag="linv", name="linv")
            nc.vector.reciprocal(linv[:], ps_av[64:65, :])
            lb = llp.tile([64, 512], F32, tag="lb", name="lb")
            nc.gpsimd.partition_broadcast(lb[:], linv[:], channels=64)
            nc.vector.tensor_mul(
                aot[g][ho:ho + 64, 512 * c:512 * (c + 1)],
                ps_av[0:64, :], lb[:])

        prev = None  # (emit_some, nblk, ps_av, g, ho, c)
        for gi, (c, h) in enumerate(groups):
            g, ho = h // 2, 64 * (h % 2)
            nblk = 4 * c + 4
            SC = SCs.pop(gi)
            if gi + LEAD < len(groups):
                SCs[gi + LEAD] = prologue(*groups[gi + LEAD])
            ps_av = psB.tile([128, 512], F32, tag="av", name="av")
            own_av = make_av_closure(SC, ps_av, h, nblk)

            stt_n = 3
            for kb in range(nblk):
                tr = max(0, 128 * kb - 512 * c)
                w = 512 - tr
                use_stt = stt_n > 0 and (kb % stt_n == stt_n - 1)
                ps = psA.tile([128, 512], F32, tag="mm", name="mm")
                nc.tensor.matmul(
                    ps[:, 0:w],
                    kt[g][ho:ho + 64, 128 * kb:128 * kb + 128],
                    qt[g][ho:ho + 64, 1 + 512 * c + tr:1 + 512 * (c + 1)],
                    start=True, stop=(True if use_stt else False))
                if use_stt:
                    nc.vector.scalar_tensor_tensor(
                        SC[:, kb, tr:512], ps[:, 0:w], 1.0, SC[:, kb, tr:512],
                        mybir.AluOpType.mult, mybir.AluOpType.add)
                    nc.scalar.activation(
                        SC[:, kb, tr:512], SC[:, kb, tr:512],
                        mybir.ActivationFunctionType.Exp, scale=0.125)
                else:
                    nc.tensor.matmul(
                        ps[:, 0:w], ident[:], SC[:, kb, tr:512],
                        start=False, stop=True)
                    nc.scalar.activation(
                        SC[:, kb, tr:512], ps[:, 0:w],
                        mybir.ActivationFunctionType.Exp, scale=0.125)
                # interleave the PREVIOUS group's AV matmuls: their exps are
                # long done, so neither side ever waits on the other
                if prev is not None:
                    pav, pnblk = prev[0], prev[1]
                    pav((kb + 1) * pnblk // nblk)
                if gi == len(groups) - 1 and kb >= 2:
                    # last group: interleave its own AVs (lag 2) to cut the tail
                    own_av(kb - 1)

            if prev is not None:
                pav, pnblk, pps_av, pg, pho, pc = prev
                pav(pnblk)
                emit_norm(pps_av, pg, pho, pc)
                if pc != c:
                    # previous group finished chunk pc: emit its out-proj
                    emit_outproj(pc)
            prev = (own_av, nblk, ps_av, g, ho, c)
        pav, pnblk, pps_av, pg, pho, pc = prev
        pav(pnblk)
        emit_norm(pps_av, pg, pho, pc)
        emit_outproj(3)
    nc.compile()
    return nc


def kernel(x, Wq, Wk, Wv, Wp, bp, rel_embed):
    x = np.asarray(x, np.float32)
    bf = ml_dtypes.bfloat16
    if "nc" not in _CACHE:
        _CACHE["nc"] = _build_nc()
    nc = _CACHE["nc"]

    in_maps = []
    for core in range(8):
        b, hg = core // 4, core % 4
        c0 = hg * DC
        wq_s = np.asarray(Wq)[c0:c0 + DC, :].T
        wk_s = np.asarray(Wk)[c0:c0 + DC, :].T
        wv_s = np.asarray(Wv)[c0:c0 + DC, :].T
        e_s = np.asarray(rel_embed)[:, c0:c0 + DC].T        # [DC, N]
        e_pad = np.concatenate([e_s, np.zeros((DC, 1), e_s.dtype)], axis=1)
        f8 = ml_dtypes.float8_e4m3

        def pack(a):
            # [D, cols] -> [kcp, p, 2, cols]
            return np.ascontiguousarray(
                a.reshape(4, 2, 128, a.shape[1]).transpose(0, 2, 1, 3))

        xT_f = np.ascontiguousarray(x[b].T).astype(np.float32)
        x1 = xT_f.astype(f8)
        x2 = (xT_f - x1.astype(np.float32)).astype(f8)
        wcat = np.concatenate([wq_s, wk_s, wv_s], axis=1).astype(np.float32)
        wcat = wcat * WSCALE
        w1 = wcat.astype(f8)
        w2 = (wcat - w1.astype(np.float32)).astype(f8)
        in_maps.append({
            "xp": np.stack([pack(x1), pack(x2)]),
            "wqkvp": np.stack([pack(w1), pack(w2)]),
            "eT": np.ascontiguousarray(e_pad).astype(bf),
            "wpT": np.ascontiguousarray(np.asarray(Wp)[:, c0:c0 + DC].T).astype(bf),
        })
    kw = dict(_CACHE.get("run_kwargs") or {})
    r = run_bass_kernel_spmd(nc, in_maps, list(range(8)), **kw)
    _CACHE["last_result"] = r
    res = r.results
    out = np.zeros((2, N, D), np.float32)
    for core in range(8):
        out[core // 4] += np.asarray(res[core]["outp"], np.float32)
    out += np.asarray(bp, np.float32)
    return out
